# revision 48
# baseline (speedup 1.0000x reference)
"""Multi-head attention (b=4, c=256, l=2048, 8 heads x 64) on 8 TRN2 NeuronCores.

Sharding: core i handles batch b = i//2 and query half qh = i%2 (1024 queries),
computing all 8 heads over the full 2048-key context. Outputs are disjoint
[256, 1024] slabs -> host-side concat only, no collectives.

Per-core kernel (all matmuls bf16, 1 cycle/row; fp32 PSUM accumulate):
  1. Q = Wq @ xq (1024 cols), K = Wk @ x (2048), VT = (Wv @ x)^T computed
     directly as x^T-stationary matmuls, laid out [l-tile 128, 8 heads x 65]
     with a ones column per head (col 64) for the softmax denominator.
  2. Per head h, per key-tile jt (16 x 128 keys):
       simT[j, i] = K_h(jt)^T . Q_h          (PSUM [128, 1024])
       E = exp(simT / 8)                     (ScalarE, PSUM -> SBUF)
       PV += VT'[jt, h]^T . E                (PSUM [65, 1024], accum over jt)
     Row 64 of PV = softmax denominator; rows 0..64 = numerator.
  3. recip = 1/PV[64] (DVE; last pair via exp(-ln) on ScalarE), partition-
     broadcast on GpSimd, attn = num * recip.
  4. out = WoutT^T . attn + bias, DMA to DRAM as bf16 (host upcasts).

Engine budget per core (measured): the kernel is DUAL-bound - ScalarE exp
stream ~143 us busy (128 ACTIVATEs @ 1114 ns) and PE ~168 us busy (QK+PV
stream 2 matmul columns per score element at 2.4 GHz vs exp's 1 element at
1.2 GHz -> both have a ~109 us floor + overheads). Wider exp tiles shift
the bottleneck to PE (measured 221 us at [128,1536]); the balanced [128,
1024] tiling with PV software-pipelined one iteration behind QK/exp holds
ACT ~97% busy mid-stream. Phase-1 DMA lead-in ~20 us, tail+postamble ~14
us -> 201-203 us total.
"""

import sys

if "/opt/trn_rl_repo" not in sys.path:
    sys.path.insert(0, "/opt/trn_rl_repo")

import numpy as np

import concourse.bass as bass
import concourse.mybir as mybir
import concourse.tile as tile
from concourse import bacc
from concourse.bass_utils import run_bass_kernel_spmd

F32 = mybir.dt.float32
F32R = mybir.dt.float32r
BF16 = mybir.dt.bfloat16
EXP = mybir.ActivationFunctionType.Exp
LOG = mybir.ActivationFunctionType.Ln
MULT = mybir.AluOpType.mult

B, C, L = 4, 256, 2048
H, D = 8, 64
HID = H * D  # 512
LQ = L // 2  # 1024 queries per core
NJT = L // 128  # 16 key tiles
SCALE = D**-0.5

_cached = {}


def r(ap):
    return ap


def build_nc():
    nc = bacc.Bacc(
        "TRN2",
        target_bir_lowering=False,
        debug=False,
        enable_asserts=False,
        num_devices=8,
    )
    x_d = nc.dram_tensor("x", [C, L], BF16, kind="ExternalInput")
    xq_d = nc.dram_tensor("xq", [C, LQ], BF16, kind="ExternalInput")
    wq_d = nc.dram_tensor("wqkvT", [C, 3 * HID], BF16, kind="ExternalInput")
    wo_d = nc.dram_tensor("woutT", [HID, C], BF16, kind="ExternalInput")
    bias_d = nc.dram_tensor("bias", [C, 1], F32, kind="ExternalInput")
    out_d = nc.dram_tensor("out", [C, LQ], BF16, kind="ExternalOutput")

    with tile.TileContext(nc) as tc:
        with (
            tc.tile_pool(name="const", bufs=1) as cp,
            tc.tile_pool(name="epool", bufs=6) as ep,
            tc.tile_pool(name="rpool", bufs=2) as rp,
            tc.tile_pool(name="opool", bufs=2) as op,
        ):
            # ---- persistent SBUF tensors ----
            xb = [cp.tile([128, L], BF16, tag=f"xb{k}", name=f"xb{k}") for k in range(2)]
            xq = [cp.tile([128, LQ], BF16, tag=f"xq{k}", name=f"xq{k}") for k in range(2)]
            wq = [cp.tile([128, 3 * HID], BF16, tag=f"wq{k}", name=f"wq{k}") for k in range(2)]
            wo = [cp.tile([128, C], BF16, tag=f"wo{k}", name=f"wo{k}") for k in range(4)]
            bias = [cp.tile([128, 1], F32, tag=f"bias{k}", name=f"bias{k}") for k in range(2)]
            Qs = [cp.tile([128, LQ], BF16, tag=f"Q{m}", name=f"Q{m}") for m in range(4)]
            Ks = [cp.tile([128, L], BF16, tag=f"K{m}", name=f"K{m}") for m in range(4)]
            VT = [cp.tile([128, H, D + 1], BF16, tag=f"VT{t}", name=f"VT{t}") for t in range(NJT)]
            attn = [cp.tile([128, LQ], BF16, tag=f"attn{m}", name=f"attn{m}") for m in range(4)]
            acc = [cp.tile([128, LQ], F32, tag=f"acc{m}", name=f"acc{m}") for m in range(2)]
            dum = cp.tile([1, 16], F32, tag="dum", name="dum")
            dumo = cp.tile([1, 16], F32, tag="dumo", name="dumo")
            nc.gpsimd.memset(dum[:], 1.0)
            nc.scalar.activation(dumo[:], dum[:], LOG)
            nc.scalar.activation(dumo[:], dum[:], EXP)

            # ---- DMA inputs ----
            nc.sync.dma_start(wq[0][:, 0:512], wq_d.ap()[0:128, 0:512])
            nc.gpsimd.dma_start(wq[1][:, 0:512], wq_d.ap()[128:256, 0:512])
            nc.sync.dma_start(wq[0][:, 512:640], wq_d.ap()[0:128, 512:640])
            nc.gpsimd.dma_start(wq[1][:, 512:640], wq_d.ap()[128:256, 512:640])
            for k in range(2):
                rows = slice(128 * k, 128 * (k + 1))
                nc.scalar.dma_start(xq[k][:, 0:512], xq_d.ap()[rows, 0:512])
            for k in range(2):
                rows = slice(128 * k, 128 * (k + 1))
                nc.scalar.dma_start(xq[k][:, 512:1024], xq_d.ap()[rows, 512:1024])
            for k in range(2):
                rows = slice(128 * k, 128 * (k + 1))
                nc.sync.dma_start(xb[k][:, 0:1024], x_d.ap()[rows, 0:1024])
            for k in range(2):
                rows = slice(128 * k, 128 * (k + 1))
                nc.sync.dma_start(wq[k][:, 1024:1536], wq_d.ap()[rows, 1024:1536])
                nc.gpsimd.dma_start(xb[k][:, 1024:2048], x_d.ap()[rows, 1024:2048])
            for k in range(2):
                rows = slice(128 * k, 128 * (k + 1))
                nc.scalar.dma_start(wq[k][:, 640:1024], wq_d.ap()[rows, 640:1024])
            for k in range(4):
                nc.sync.dma_start(wo[k][:], wo_d.ap()[128 * k : 128 * (k + 1), :])
            for k in range(2):
                rows = slice(128 * k, 128 * (k + 1))
                nc.gpsimd.dma_start(bias[k][:], bias_d.ap()[rows, :])

            # ---- phases 1+2 fused: projections share the segment psum
            # pools (a [128,1024] projection tile fits a 3-bank qk slot), so
            # there is no pool-drain barrier and segment 0 starts as soon as
            # Q0/K0 exist. VT projections are emitted just-in-time inside
            # segment 0's loop; Q1-3/K1-3 are emitted at the next three
            # segment boundaries, where their DMAs have long since landed.
            with (
                tc.tile_pool(name="qkps", bufs=2, space=bass.MemorySpace.PSUM) as qkps,
                tc.tile_pool(name="pvps", bufs=1, space=bass.MemorySpace.PSUM) as pvps,
            ):
                def q_proj(m):
                    ps = qkps.tile([128, LQ], F32, tag="qk", name="ps")
                    for k in range(2):
                        for n in range(2):
                            nc.tensor.matmul(
                                ps[:, 512 * n : 512 * (n + 1)],
                                wq[k][:, 128 * m : 128 * (m + 1)],
                                xq[k][:, 512 * n : 512 * (n + 1)],
                                start=(k == 0),
                                stop=(k == 1),
                            )
                    if m == 0:
                        for n in range(2):
                            cols = slice(512 * n, 512 * (n + 1))
                            nc.vector.tensor_scalar_mul(
                                Qs[m][:, cols], ps[:, cols], SCALE
                            )
                    else:
                        nc.scalar.mul(Qs[m][:], ps[:], SCALE)

                def k_proj(m):
                    for lh in range(2):
                        ps = qkps.tile([128, LQ], F32, tag="qk", name="ps")
                        for k in range(2):
                            for n in range(2):
                                nc.tensor.matmul(
                                    ps[:, 512 * n : 512 * (n + 1)],
                                    wq[k][:, HID + 128 * m : HID + 128 * (m + 1)],
                                    xb[k][:, 1024 * lh + 512 * n : 1024 * lh + 512 * (n + 1)],
                                    start=(k == 0),
                                    stop=(k == 1),
                                )
                        if m == 0:
                            for n in range(2):
                                nc.scalar.copy(
                                    Ks[m][
                                        :,
                                        1024 * lh + 512 * n : 1024 * lh + 512 * (n + 1),
                                    ],
                                    ps[:, 512 * n : 512 * (n + 1)],
                                )
                        else:
                            nc.vector.tensor_copy(
                                Ks[m][:, 1024 * lh : 1024 * (lh + 1)], ps[:]
                            )

                def vt_proj(t):
                    ps = qkps.tile([128, HID], F32, tag="qk", name="psv")
                    for k in range(2):
                        nc.tensor.matmul(
                            ps[:],
                            xb[k][:, 128 * t : 128 * (t + 1)],
                            wq[k][:, 2 * HID : 3 * HID],
                            start=(k == 0),
                            stop=(k == 1),
                        )
                    nc.vector.tensor_copy(
                        VT[t][:, :, 0:D], ps[:].rearrange("p (h c) -> p h c", h=H)
                    )
                    nc.gpsimd.memset(VT[t][:, :, D : D + 1], 1.0)

                def do_norm(pons, p, ih, on_act):
                    cols = slice(512 * ih, 512 * (ih + 1))
                    for s in (0, 1):
                        rec = rp.tile([1, 512], F32, tag="rec", name="rec")
                        if on_act:
                            lnd = rp.tile([1, 512], F32, tag="lnd", name="lnd")
                            nc.scalar.activation(lnd[:], pons[s][D : D + 1, :], LOG)
                            nc.scalar.activation(rec[:], lnd[:], EXP, scale=-1.0)
                        else:
                            nc.vector.reciprocal(rec[:], pons[s][D : D + 1, :])
                        rbc = rp.tile([64, 512], F32, tag="rbc", name="rbc")
                        nc.gpsimd.partition_broadcast(rbc[:], rec[:])
                        nc.vector.tensor_tensor(
                            attn[p][64 * s : 64 * (s + 1), cols],
                            pons[s][0:D, :],
                            rbc[:],
                            MULT,
                        )

                q_proj(0)
                k_proj(0)
                vt_done = [0]
                prev_norm = None
                for seg in range(8):
                    p, ih = divmod(seg, 2)
                    Qh = [
                        Qs[p][64 * s : 64 * (s + 1), 512 * ih : 512 * (ih + 1)]
                        for s in (0, 1)
                    ]
                    Kh = [Ks[p][64 * s : 64 * (s + 1), :] for s in (0, 1)]
                    po = [
                        pvps.tile([D + 1, 512], F32, tag=f"pv{s}", name=f"po{s}")
                        for s in (0, 1)
                    ]

                    def pv_flush(E, c0, nch, po=po, p=p):
                        for ci in range(nch):
                            jt, s = divmod(c0 + ci, 2)
                            nc.tensor.matmul(
                                po[s][:],
                                VT[jt][:, 2 * p + s, :],
                                E[:, 512 * ci : 512 * (ci + 1)],
                                start=(jt == 0),
                                stop=(jt == NJT - 1),
                            )

                    prev = None
                    c0 = 0
                    ti = 0
                    while c0 < 2 * NJT:
                        nch = min(3, 2 * NJT - c0)
                        ps = qkps.tile([128, 512 * nch], F32, tag="qk", name="psqk")
                        for ci in range(nch):
                            jt, s = divmod(c0 + ci, 2)
                            nc.tensor.matmul(
                                ps[:, 512 * ci : 512 * (ci + 1)],
                                Kh[s][:, 128 * jt : 128 * (jt + 1)],
                                Qh[s][:],
                                start=True,
                                stop=True,
                            )
                        if seg == 0:
                            # just-in-time VT projections, 2 per score tile
                            target = min(NJT, 2 * (ti + 1))
                            while vt_done[0] < target:
                                vt_proj(vt_done[0])
                                vt_done[0] += 1
                        if prev is not None:
                            pv_flush(*prev)
                        E = ep.tile([128, 512 * nch], BF16, tag="e", name="E")
                        nc.scalar.activation(E[:], ps[:], EXP)
                        prev = (E, c0, nch)
                        c0 += nch
                        ti += 1
                    pv_flush(*prev)

                    pons = [
                        rp.tile([D + 1, 512], F32, tag="pon", name="pon", bufs=4)
                        for s in (0, 1)
                    ]
                    for s in (0, 1):
                        nc.vector.tensor_copy(pons[s][:], po[s][:])
                    if prev_norm is not None:
                        do_norm(*prev_norm, on_act=False)
                    prev_norm = (pons, p, ih)
                    if seg < 3:
                        q_proj(seg + 1)
                        k_proj(seg + 1)
                do_norm(*prev_norm, on_act=True)

            # ---- phase 3: output projection ----
            with tc.tile_pool(name="ops", bufs=2, space=bass.MemorySpace.PSUM) as ops:
                for m in range(2):
                    ps = ops.tile([128, LQ], F32, tag="o", name="pso")
                    for k in range(3):
                        for n in range(2):
                            nc.tensor.matmul(
                                ps[:, 512 * n : 512 * (n + 1)],
                                wo[k][:, 128 * m : 128 * (m + 1)],
                                attn[k][:, 512 * n : 512 * (n + 1)],
                                start=(k == 0),
                                stop=False,
                            )
                    for half in range(2):
                        hr = slice(64 * half, 64 * (half + 1))
                        for n in range(2):
                            nc.tensor.matmul(
                                ps[:, 512 * n : 512 * (n + 1)],
                                wo[3][hr, 128 * m : 128 * (m + 1)],
                                attn[3][hr, 512 * n : 512 * (n + 1)],
                                start=False,
                                stop=(half == 1),
                            )
                    osb = op.tile([128, LQ], BF16, tag="osb", name="osb")
                    for n in range(2):
                        cols = slice(512 * n, 512 * (n + 1))
                        if m == 0:
                            nc.scalar.add(osb[:, cols], ps[:, cols], bias[m][:])
                        else:
                            nc.vector.tensor_scalar_add(
                                osb[:, cols], ps[:, cols], bias[m][:]
                            )
                        nc.sync.dma_start(
                            out_d.ap()[128 * m : 128 * (m + 1), cols], osb[:, cols]
                        )

    nc.compile()
    return nc


def get_nc():
    if "nc" not in _cached:
        _cached["nc"] = build_nc()
    return _cached["nc"]


def make_in_maps(x, w_qkv, w_out, b_out):
    import ml_dtypes

    bf16 = ml_dtypes.bfloat16
    wqkvT = np.ascontiguousarray(w_qkv.T.astype(bf16))
    woutT = np.ascontiguousarray(w_out.T.astype(bf16))
    bias = np.ascontiguousarray(b_out.astype(np.float32).reshape(C, 1))
    in_maps = []
    for i in range(8):
        b, qh = i // 2, i % 2
        xb = np.ascontiguousarray(x[b].astype(bf16))
        xq = np.ascontiguousarray(xb[:, qh * LQ : (qh + 1) * LQ])
        in_maps.append(
            {"x": xb, "xq": xq, "wqkvT": wqkvT, "woutT": woutT, "bias": bias}
        )
    return in_maps


def assemble(results):
    out = np.empty((B, C, L), dtype=np.float32)
    for i in range(8):
        b, qh = i // 2, i % 2
        out[b][:, qh * LQ : (qh + 1) * LQ] = np.asarray(
            results[i]["out"], dtype=np.float32
        )
    return out


def kernel(x, w_qkv, w_out, b_out):
    x = np.asarray(x, dtype=np.float32)
    w_qkv = np.asarray(w_qkv, dtype=np.float32)
    w_out = np.asarray(w_out, dtype=np.float32)
    b_out = np.asarray(b_out, dtype=np.float32)
    assert x.shape == (B, C, L), x.shape
    nc = get_nc()
    in_maps = make_in_maps(x, w_qkv, w_out, b_out)
    res = run_bass_kernel_spmd(nc, in_maps, list(range(8)), trace=False)
    return assemble(res.results)


# revision 49
# speedup vs baseline: 1.0860x; 1.0860x over previous
"""Multi-head attention (b=4, c=256, l=2048, 8 heads x 64) on 8 TRN2 NeuronCores.

Sharding: core i handles batch b = i//2 and query half qh = i%2 (1024 queries),
computing all 8 heads over the full 2048-key context. Outputs are disjoint
[256, 1024] slabs -> host-side concat only, no collectives.

Per-core kernel (all matmuls bf16, 1 cycle/row; fp32 PSUM accumulate):
  1. Q = Wq @ xq (1024 cols), K = Wk @ x (2048), VT = (Wv @ x)^T computed
     directly as x^T-stationary matmuls, laid out [l-tile 128, 8 heads x 65]
     with a ones column per head (col 64) for the softmax denominator.
  2. Per head h, per key-tile jt (16 x 128 keys):
       simT[j, i] = K_h(jt)^T . Q_h          (PSUM [128, 1024])
       E = exp(simT / 8)                     (ScalarE, PSUM -> SBUF)
       PV += VT'[jt, h]^T . E                (PSUM [65, 1024], accum over jt)
     Row 64 of PV = softmax denominator; rows 0..64 = numerator.
  3. recip = 1/PV[64] (DVE; last pair via exp(-ln) on ScalarE), partition-
     broadcast on GpSimd, attn = num * recip.
  4. out = WoutT^T . attn + bias, DMA to DRAM as bf16 (host upcasts).

Engine budget per core (measured): the kernel is DUAL-bound - ScalarE exp
stream ~143 us busy (128 ACTIVATEs @ 1114 ns) and PE ~168 us busy (QK+PV
stream 2 matmul columns per score element at 2.4 GHz vs exp's 1 element at
1.2 GHz -> both have a ~109 us floor + overheads). Wider exp tiles shift
the bottleneck to PE (measured 221 us at [128,1536]); the balanced [128,
1024] tiling with PV software-pipelined one iteration behind QK/exp holds
ACT ~97% busy mid-stream. Phase-1 DMA lead-in ~20 us, tail+postamble ~14
us -> 201-203 us total.
"""

import sys

if "/opt/trn_rl_repo" not in sys.path:
    sys.path.insert(0, "/opt/trn_rl_repo")

import numpy as np

import concourse.bass as bass
import concourse.mybir as mybir
import concourse.tile as tile
from concourse import bacc
from concourse.bass_utils import run_bass_kernel_spmd

F32 = mybir.dt.float32
F32R = mybir.dt.float32r
BF16 = mybir.dt.bfloat16
EXP = mybir.ActivationFunctionType.Exp
LOG = mybir.ActivationFunctionType.Ln
MULT = mybir.AluOpType.mult

B, C, L = 4, 256, 2048
H, D = 8, 64
HID = H * D  # 512
LQ = L // 2  # 1024 queries per core
NJT = L // 128  # 16 key tiles
SCALE = D**-0.5

_cached = {}


def r(ap):
    return ap


def build_nc():
    nc = bacc.Bacc(
        "TRN2",
        target_bir_lowering=False,
        debug=False,
        enable_asserts=False,
        num_devices=8,
    )
    x_d = nc.dram_tensor("x", [C, L], BF16, kind="ExternalInput")
    xq_d = nc.dram_tensor("xq", [C, LQ], BF16, kind="ExternalInput")
    wq_d = nc.dram_tensor("wqkvT", [C, 3 * HID], BF16, kind="ExternalInput")
    wo_d = nc.dram_tensor("woutT", [HID, C], BF16, kind="ExternalInput")
    bias_d = nc.dram_tensor("bias", [C, 1], F32, kind="ExternalInput")
    out_d = nc.dram_tensor("out", [C, LQ], BF16, kind="ExternalOutput")

    with tile.TileContext(nc) as tc:
        with (
            tc.tile_pool(name="const", bufs=1) as cp,
            tc.tile_pool(name="epool", bufs=6) as ep,
            tc.tile_pool(name="rpool", bufs=2) as rp,
            tc.tile_pool(name="opool", bufs=2) as op,
        ):
            # ---- persistent SBUF tensors ----
            xb = [cp.tile([128, L], BF16, tag=f"xb{k}", name=f"xb{k}") for k in range(2)]
            xq = [cp.tile([128, LQ], BF16, tag=f"xq{k}", name=f"xq{k}") for k in range(2)]
            wq = [cp.tile([128, 3 * HID], BF16, tag=f"wq{k}", name=f"wq{k}") for k in range(2)]
            wo = [cp.tile([128, C], BF16, tag=f"wo{k}", name=f"wo{k}") for k in range(4)]
            bias = [cp.tile([128, 1], F32, tag=f"bias{k}", name=f"bias{k}") for k in range(2)]
            Qs = [cp.tile([128, LQ], BF16, tag=f"Q{m}", name=f"Q{m}") for m in range(4)]
            Ks = [cp.tile([128, L], BF16, tag=f"K{m}", name=f"K{m}") for m in range(4)]
            VT = [cp.tile([128, H, D + 1], BF16, tag=f"VT{t}", name=f"VT{t}") for t in range(NJT)]
            attn = [cp.tile([128, LQ], BF16, tag=f"attn{m}", name=f"attn{m}") for m in range(4)]
            acc = [cp.tile([128, LQ], F32, tag=f"acc{m}", name=f"acc{m}") for m in range(2)]
            dum = cp.tile([1, 16], F32, tag="dum", name="dum")
            dumo = cp.tile([1, 16], F32, tag="dumo", name="dumo")
            nc.gpsimd.memset(dum[:], 1.0)
            nc.scalar.activation(dumo[:], dum[:], LOG)
            nc.scalar.activation(dumo[:], dum[:], EXP)

            # ---- DMA inputs ----
            nc.sync.dma_start(wq[0][:, 0:512], wq_d.ap()[0:128, 0:512])
            nc.gpsimd.dma_start(wq[1][:, 0:512], wq_d.ap()[128:256, 0:512])
            nc.sync.dma_start(wq[0][:, 512:640], wq_d.ap()[0:128, 512:640])
            nc.gpsimd.dma_start(wq[1][:, 512:640], wq_d.ap()[128:256, 512:640])
            for k in range(2):
                rows = slice(128 * k, 128 * (k + 1))
                nc.scalar.dma_start(xq[k][:, 0:512], xq_d.ap()[rows, 0:512])
            for k in range(2):
                rows = slice(128 * k, 128 * (k + 1))
                nc.scalar.dma_start(xq[k][:, 512:1024], xq_d.ap()[rows, 512:1024])
            for k in range(2):
                rows = slice(128 * k, 128 * (k + 1))
                nc.sync.dma_start(xb[k][:, 0:1024], x_d.ap()[rows, 0:1024])
            for k in range(2):
                rows = slice(128 * k, 128 * (k + 1))
                nc.scalar.dma_start(wq[k][:, 640:1536], wq_d.ap()[rows, 640:1536])
                nc.gpsimd.dma_start(xb[k][:, 1024:2048], x_d.ap()[rows, 1024:2048])
            for k in range(4):
                nc.sync.dma_start(wo[k][:], wo_d.ap()[128 * k : 128 * (k + 1), :])
            for k in range(2):
                rows = slice(128 * k, 128 * (k + 1))
                nc.gpsimd.dma_start(bias[k][:], bias_d.ap()[rows, :])

            # ---- phase 1: projections ----
            with (
                tc.tile_pool(name="pps", bufs=2, space=bass.MemorySpace.PSUM) as pps,
                tc.tile_pool(name="vps", bufs=2, space=bass.MemorySpace.PSUM) as vps,
            ):
                def q_proj(m):
                    ps = pps.tile([128, LQ], F32, tag="proj", name="ps")
                    for k in range(2):
                        for n in range(2):
                            nc.tensor.matmul(
                                ps[:, 512 * n : 512 * (n + 1)],
                                wq[k][:, 128 * m : 128 * (m + 1)],
                                xq[k][:, 512 * n : 512 * (n + 1)],
                                start=(k == 0),
                                stop=(k == 1),
                            )
                    if m == 0:
                        for n in range(2):
                            cols = slice(512 * n, 512 * (n + 1))
                            nc.vector.tensor_scalar_mul(
                                Qs[m][:, cols], ps[:, cols], SCALE
                            )
                    else:
                        nc.scalar.mul(Qs[m][:], ps[:], SCALE)

                def k_proj(m):
                    for lh in range(2):
                        ps = pps.tile([128, LQ], F32, tag="proj", name="ps")
                        for k in range(2):
                            for n in range(2):
                                nc.tensor.matmul(
                                    ps[:, 512 * n : 512 * (n + 1)],
                                    wq[k][:, HID + 128 * m : HID + 128 * (m + 1)],
                                    xb[k][:, 1024 * lh + 512 * n : 1024 * lh + 512 * (n + 1)],
                                    start=(k == 0),
                                    stop=(k == 1),
                                )
                        if m == 0:
                            for n in range(2):
                                nc.scalar.copy(
                                    Ks[m][
                                        :,
                                        1024 * lh + 512 * n : 1024 * lh + 512 * (n + 1),
                                    ],
                                    ps[:, 512 * n : 512 * (n + 1)],
                                )
                        else:
                            nc.vector.tensor_copy(
                                Ks[m][:, 1024 * lh : 1024 * (lh + 1)], ps[:]
                            )

                def vt_proj(t):
                    ps = vps.tile([128, HID], F32, tag="vproj", name="psv")
                    for k in range(2):
                        nc.tensor.matmul(
                            ps[:],
                            xb[k][:, 128 * t : 128 * (t + 1)],
                            wq[k][:, 2 * HID : 3 * HID],
                            start=(k == 0),
                            stop=(k == 1),
                        )
                    nc.vector.tensor_copy(
                        VT[t][:, :, 0:D], ps[:].rearrange("p (h c) -> p h c", h=H)
                    )
                    nc.gpsimd.memset(VT[t][:, :, D : D + 1], 1.0)

                q_proj(0)
                k_proj(0)
                vt_proj(0)
                vt_proj(1)
                q_proj(1)
                k_proj(1)
                vt_proj(2)
                vt_proj(3)
                q_proj(2)
                k_proj(2)
                q_proj(3)
                k_proj(3)
                for t in range(4, NJT):
                    vt_proj(t)

            # ---- phase 2: attention ----
            # Segments = (head-pair, 512-query-half). Scores go to [128,
            # 1536] psum tiles (3 chunks of 512 cols, both heads interleaved
            # - exp is elementwise so mixed-head tiles are fine). This
            # amortizes the ACTIVATE overhead (1540 ns / 3 chunks vs 1114/2)
            # while the PV accumulators shrink to [65, 512] = 1 bank each:
            # qk 2x3 banks + pv 2x1 = 8 banks exactly. PV lags one tile; the
            # normalization chain runs one SEGMENT behind so the PV slots
            # free after two fast copies.
            with (
                tc.tile_pool(name="qkps", bufs=2, space=bass.MemorySpace.PSUM) as qkps,
                tc.tile_pool(name="pvps", bufs=1, space=bass.MemorySpace.PSUM) as pvps,
            ):
                def do_norm(pons, p, ih, on_act):
                    cols = slice(512 * ih, 512 * (ih + 1))
                    for s in (0, 1):
                        rec = rp.tile([1, 512], F32, tag="rec", name="rec")
                        if on_act:
                            lnd = rp.tile([1, 512], F32, tag="lnd", name="lnd")
                            nc.scalar.activation(lnd[:], pons[s][D : D + 1, :], LOG)
                            nc.scalar.activation(rec[:], lnd[:], EXP, scale=-1.0)
                        else:
                            nc.vector.reciprocal(rec[:], pons[s][D : D + 1, :])
                        rbc = rp.tile([64, 512], F32, tag="rbc", name="rbc")
                        nc.gpsimd.partition_broadcast(rbc[:], rec[:])
                        nc.vector.tensor_tensor(
                            attn[p][64 * s : 64 * (s + 1), cols],
                            pons[s][0:D, :],
                            rbc[:],
                            MULT,
                        )

                prev_norm = None
                for seg in range(8):
                    p, ih = divmod(seg, 2)
                    Qh = [
                        Qs[p][64 * s : 64 * (s + 1), 512 * ih : 512 * (ih + 1)]
                        for s in (0, 1)
                    ]
                    Kh = [Ks[p][64 * s : 64 * (s + 1), :] for s in (0, 1)]
                    po = [
                        pvps.tile([D + 1, 512], F32, tag=f"pv{s}", name=f"po{s}")
                        for s in (0, 1)
                    ]

                    def pv_flush(E, c0, nch, po=po, p=p):
                        for ci in range(nch):
                            jt, s = divmod(c0 + ci, 2)
                            nc.tensor.matmul(
                                po[s][:],
                                VT[jt][:, 2 * p + s, :],
                                E[:, 512 * ci : 512 * (ci + 1)],
                                start=(jt == 0),
                                stop=(jt == NJT - 1),
                            )

                    prev = None
                    c0 = 0
                    while c0 < 2 * NJT:
                        nch = min(3, 2 * NJT - c0)
                        ps = qkps.tile([128, 512 * nch], F32, tag="qk", name="psqk")
                        for ci in range(nch):
                            jt, s = divmod(c0 + ci, 2)
                            nc.tensor.matmul(
                                ps[:, 512 * ci : 512 * (ci + 1)],
                                Kh[s][:, 128 * jt : 128 * (jt + 1)],
                                Qh[s][:],
                                start=True,
                                stop=True,
                            )
                        if prev is not None:
                            pv_flush(*prev)
                        E = ep.tile([128, 512 * nch], BF16, tag="e", name="E")
                        nc.scalar.activation(E[:], ps[:], EXP)
                        prev = (E, c0, nch)
                        c0 += nch
                    pv_flush(*prev)

                    pons = [
                        rp.tile([D + 1, 512], F32, tag="pon", name="pon", bufs=4)
                        for s in (0, 1)
                    ]
                    for s in (0, 1):
                        nc.vector.tensor_copy(pons[s][:], po[s][:])
                    if prev_norm is not None:
                        do_norm(*prev_norm, on_act=False)
                    prev_norm = (pons, p, ih)
                do_norm(*prev_norm, on_act=True)

            # ---- phase 3: output projection ----
            with tc.tile_pool(name="ops", bufs=2, space=bass.MemorySpace.PSUM) as ops:
                for m in range(2):
                    ps = ops.tile([128, LQ], F32, tag="o", name="pso")
                    for k in range(3):
                        for n in range(2):
                            nc.tensor.matmul(
                                ps[:, 512 * n : 512 * (n + 1)],
                                wo[k][:, 128 * m : 128 * (m + 1)],
                                attn[k][:, 512 * n : 512 * (n + 1)],
                                start=(k == 0),
                                stop=False,
                            )
                    for half in range(2):
                        hr = slice(64 * half, 64 * (half + 1))
                        for n in range(2):
                            nc.tensor.matmul(
                                ps[:, 512 * n : 512 * (n + 1)],
                                wo[3][hr, 128 * m : 128 * (m + 1)],
                                attn[3][hr, 512 * n : 512 * (n + 1)],
                                start=False,
                                stop=(half == 1),
                            )
                    osb = op.tile([128, LQ], BF16, tag="osb", name="osb")
                    for n in range(2):
                        cols = slice(512 * n, 512 * (n + 1))
                        if m == 0:
                            nc.scalar.add(osb[:, cols], ps[:, cols], bias[m][:])
                        else:
                            nc.vector.tensor_scalar_add(
                                osb[:, cols], ps[:, cols], bias[m][:]
                            )
                        nc.sync.dma_start(
                            out_d.ap()[128 * m : 128 * (m + 1), cols], osb[:, cols]
                        )

    nc.compile()
    return nc


def get_nc():
    if "nc" not in _cached:
        _cached["nc"] = build_nc()
    return _cached["nc"]


def make_in_maps(x, w_qkv, w_out, b_out):
    import ml_dtypes

    bf16 = ml_dtypes.bfloat16
    wqkvT = np.ascontiguousarray(w_qkv.T.astype(bf16))
    woutT = np.ascontiguousarray(w_out.T.astype(bf16))
    bias = np.ascontiguousarray(b_out.astype(np.float32).reshape(C, 1))
    in_maps = []
    for i in range(8):
        b, qh = i // 2, i % 2
        xb = np.ascontiguousarray(x[b].astype(bf16))
        xq = np.ascontiguousarray(xb[:, qh * LQ : (qh + 1) * LQ])
        in_maps.append(
            {"x": xb, "xq": xq, "wqkvT": wqkvT, "woutT": woutT, "bias": bias}
        )
    return in_maps


def assemble(results):
    out = np.empty((B, C, L), dtype=np.float32)
    for i in range(8):
        b, qh = i // 2, i % 2
        out[b][:, qh * LQ : (qh + 1) * LQ] = np.asarray(
            results[i]["out"], dtype=np.float32
        )
    return out


def kernel(x, w_qkv, w_out, b_out):
    x = np.asarray(x, dtype=np.float32)
    w_qkv = np.asarray(w_qkv, dtype=np.float32)
    w_out = np.asarray(w_out, dtype=np.float32)
    b_out = np.asarray(b_out, dtype=np.float32)
    assert x.shape == (B, C, L), x.shape
    nc = get_nc()
    in_maps = make_in_maps(x, w_qkv, w_out, b_out)
    res = run_bass_kernel_spmd(nc, in_maps, list(range(8)), trace=False)
    return assemble(res.results)


# revision 50
# speedup vs baseline: 1.1043x; 1.0168x over previous
"""Multi-head attention (b=4, c=256, l=2048, 8 heads x 64) on 8 TRN2 NeuronCores.

Sharding: core i handles batch b = i//2 and query half qh = i%2 (1024 queries),
computing all 8 heads over the full 2048-key context. Outputs are disjoint
[256, 1024] slabs -> host-side concat only, no collectives.

Per-core kernel (all matmuls bf16, 1 cycle/row; fp32 PSUM accumulate):
  1. Q = Wq @ xq (1024 cols), K = Wk @ x (2048), VT = (Wv @ x)^T computed
     directly as x^T-stationary matmuls, laid out [l-tile 128, 8 heads x 65]
     with a ones column per head (col 64) for the softmax denominator.
  2. Per head h, per key-tile jt (16 x 128 keys):
       simT[j, i] = K_h(jt)^T . Q_h          (PSUM [128, 1024])
       E = exp(simT / 8)                     (ScalarE, PSUM -> SBUF)
       PV += VT'[jt, h]^T . E                (PSUM [65, 1024], accum over jt)
     Row 64 of PV = softmax denominator; rows 0..64 = numerator.
  3. recip = 1/PV[64] (DVE; last pair via exp(-ln) on ScalarE), partition-
     broadcast on GpSimd, attn = num * recip.
  4. out = WoutT^T . attn + bias, DMA to DRAM as bf16 (host upcasts).

Engine budget per core (measured): the kernel is DUAL-bound - ScalarE exp
stream ~143 us busy (128 ACTIVATEs @ 1114 ns) and PE ~168 us busy (QK+PV
stream 2 matmul columns per score element at 2.4 GHz vs exp's 1 element at
1.2 GHz -> both have a ~109 us floor + overheads). Wider exp tiles shift
the bottleneck to PE (measured 221 us at [128,1536]); the balanced [128,
1024] tiling with PV software-pipelined one iteration behind QK/exp holds
ACT ~97% busy mid-stream. Phase-1 DMA lead-in ~20 us, tail+postamble ~14
us -> 201-203 us total.
"""

import sys

if "/opt/trn_rl_repo" not in sys.path:
    sys.path.insert(0, "/opt/trn_rl_repo")

import numpy as np

import concourse.bass as bass
import concourse.mybir as mybir
import concourse.tile as tile
from concourse import bacc
from concourse.bass_utils import run_bass_kernel_spmd

F32 = mybir.dt.float32
F32R = mybir.dt.float32r
BF16 = mybir.dt.bfloat16
EXP = mybir.ActivationFunctionType.Exp
LOG = mybir.ActivationFunctionType.Ln
MULT = mybir.AluOpType.mult

B, C, L = 4, 256, 2048
H, D = 8, 64
HID = H * D  # 512
LQ = L // 2  # 1024 queries per core
NJT = L // 128  # 16 key tiles
SCALE = D**-0.5

_cached = {}


def r(ap):
    return ap


def build_nc():
    nc = bacc.Bacc(
        "TRN2",
        target_bir_lowering=False,
        debug=False,
        enable_asserts=False,
        num_devices=8,
    )
    x_d = nc.dram_tensor("x", [C, L], BF16, kind="ExternalInput")
    xq_d = nc.dram_tensor("xq", [C, LQ], BF16, kind="ExternalInput")
    wq_d = nc.dram_tensor("wqkvT", [C, 3 * HID], BF16, kind="ExternalInput")
    wo_d = nc.dram_tensor("woutT", [HID, C], BF16, kind="ExternalInput")
    bias_d = nc.dram_tensor("bias", [C, 1], F32, kind="ExternalInput")
    out_d = nc.dram_tensor("out", [C, LQ], BF16, kind="ExternalOutput")

    with tile.TileContext(nc) as tc:
        with (
            tc.tile_pool(name="const", bufs=1) as cp,
            tc.tile_pool(name="epool", bufs=6) as ep,
            tc.tile_pool(name="rpool", bufs=2) as rp,
            tc.tile_pool(name="opool", bufs=2) as op,
        ):
            # ---- persistent SBUF tensors ----
            xb = [cp.tile([128, L], BF16, tag=f"xb{k}", name=f"xb{k}") for k in range(2)]
            xq = [cp.tile([128, LQ], BF16, tag=f"xq{k}", name=f"xq{k}") for k in range(2)]
            wq = [cp.tile([128, 3 * HID], BF16, tag=f"wq{k}", name=f"wq{k}") for k in range(2)]
            wo = [cp.tile([128, C], BF16, tag=f"wo{k}", name=f"wo{k}") for k in range(4)]
            bias = [cp.tile([128, 1], F32, tag=f"bias{k}", name=f"bias{k}") for k in range(2)]
            Qs = [cp.tile([128, LQ], BF16, tag=f"Q{m}", name=f"Q{m}") for m in range(4)]
            Ks = [cp.tile([128, L], BF16, tag=f"K{m}", name=f"K{m}") for m in range(4)]
            VT = [cp.tile([128, H, D + 1], BF16, tag=f"VT{t}", name=f"VT{t}") for t in range(NJT)]
            attn = [cp.tile([128, LQ], BF16, tag=f"attn{m}", name=f"attn{m}") for m in range(4)]
            acc = [cp.tile([128, LQ], F32, tag=f"acc{m}", name=f"acc{m}") for m in range(2)]
            dum = cp.tile([1, 16], F32, tag="dum", name="dum")
            dumo = cp.tile([1, 16], F32, tag="dumo", name="dumo")
            nc.gpsimd.memset(dum[:], 1.0)
            nc.scalar.activation(dumo[:], dum[:], LOG)
            nc.scalar.activation(dumo[:], dum[:], EXP)

            # ---- DMA inputs ----
            nc.sync.dma_start(wq[0][:, 0:512], wq_d.ap()[0:128, 0:512])
            nc.gpsimd.dma_start(wq[1][:, 0:512], wq_d.ap()[128:256, 0:512])
            nc.sync.dma_start(wq[0][:, 512:640], wq_d.ap()[0:128, 512:640])
            nc.gpsimd.dma_start(wq[1][:, 512:640], wq_d.ap()[128:256, 512:640])
            for k in range(2):
                rows = slice(128 * k, 128 * (k + 1))
                nc.scalar.dma_start(xq[k][:, 0:512], xq_d.ap()[rows, 0:512])
            for k in range(2):
                rows = slice(128 * k, 128 * (k + 1))
                nc.scalar.dma_start(xq[k][:, 512:1024], xq_d.ap()[rows, 512:1024])
            for k in range(2):
                rows = slice(128 * k, 128 * (k + 1))
                nc.sync.dma_start(xb[k][:, 0:1024], x_d.ap()[rows, 0:1024])
            for k in range(2):
                rows = slice(128 * k, 128 * (k + 1))
                nc.scalar.dma_start(wq[k][:, 640:1536], wq_d.ap()[rows, 640:1536])
                nc.gpsimd.dma_start(xb[k][:, 1024:2048], x_d.ap()[rows, 1024:2048])
            for k in range(4):
                nc.sync.dma_start(wo[k][:], wo_d.ap()[128 * k : 128 * (k + 1), :])
            for k in range(2):
                rows = slice(128 * k, 128 * (k + 1))
                nc.gpsimd.dma_start(bias[k][:], bias_d.ap()[rows, :])

            # ---- phase 1: projections ----
            with (
                tc.tile_pool(name="pps", bufs=2, space=bass.MemorySpace.PSUM) as pps,
                tc.tile_pool(name="vps", bufs=2, space=bass.MemorySpace.PSUM) as vps,
            ):
                def q_proj(m):
                    ps = pps.tile([128, LQ], F32, tag="proj", name="ps")
                    for k in range(2):
                        for n in range(2):
                            nc.tensor.matmul(
                                ps[:, 512 * n : 512 * (n + 1)],
                                wq[k][:, 128 * m : 128 * (m + 1)],
                                xq[k][:, 512 * n : 512 * (n + 1)],
                                start=(k == 0),
                                stop=(k == 1),
                            )
                    if m == 0:
                        for n in range(2):
                            cols = slice(512 * n, 512 * (n + 1))
                            nc.vector.tensor_scalar_mul(
                                Qs[m][:, cols], ps[:, cols], SCALE
                            )
                    else:
                        nc.scalar.mul(Qs[m][:], ps[:], SCALE)

                def k_proj(m):
                    for lh in range(2):
                        ps = pps.tile([128, LQ], F32, tag="proj", name="ps")
                        for k in range(2):
                            for n in range(2):
                                nc.tensor.matmul(
                                    ps[:, 512 * n : 512 * (n + 1)],
                                    wq[k][:, HID + 128 * m : HID + 128 * (m + 1)],
                                    xb[k][:, 1024 * lh + 512 * n : 1024 * lh + 512 * (n + 1)],
                                    start=(k == 0),
                                    stop=(k == 1),
                                )
                        if m == 0:
                            for n in range(2):
                                nc.scalar.copy(
                                    Ks[m][
                                        :,
                                        1024 * lh + 512 * n : 1024 * lh + 512 * (n + 1),
                                    ],
                                    ps[:, 512 * n : 512 * (n + 1)],
                                )
                        else:
                            nc.vector.tensor_copy(
                                Ks[m][:, 1024 * lh : 1024 * (lh + 1)], ps[:]
                            )

                def vt_proj(t):
                    ps = vps.tile([128, HID], F32, tag="vproj", name="psv")
                    for k in range(2):
                        nc.tensor.matmul(
                            ps[:],
                            xb[k][:, 128 * t : 128 * (t + 1)],
                            wq[k][:, 2 * HID : 3 * HID],
                            start=(k == 0),
                            stop=(k == 1),
                        )
                    nc.vector.tensor_copy(
                        VT[t][:, :, 0:D], ps[:].rearrange("p (h c) -> p h c", h=H)
                    )
                    nc.gpsimd.memset(VT[t][:, :, D : D + 1], 1.0)

                q_proj(0)
                k_proj(0)
                vt_proj(0)
                vt_proj(1)
                q_proj(1)
                k_proj(1)
                vt_proj(2)
                vt_proj(3)
                q_proj(2)
                k_proj(2)
                q_proj(3)
                k_proj(3)
                for t in range(4, NJT):
                    vt_proj(t)

            # ---- phase 2: attention ----
            # Segments = (head-pair, 512-query-half). Scores go to [128,
            # 1536] psum tiles (3 chunks of 512 cols, both heads interleaved
            # - exp is elementwise so mixed-head tiles are fine). This
            # amortizes the ACTIVATE overhead (1540 ns / 3 chunks vs 1114/2)
            # while the PV accumulators shrink to [65, 512] = 1 bank each:
            # qk 2x3 banks + pv 2x1 = 8 banks exactly. PV lags one tile; the
            # normalization chain runs one SEGMENT behind so the PV slots
            # free after two fast copies.
            with (
                tc.tile_pool(name="qkps", bufs=2, space=bass.MemorySpace.PSUM) as qkps,
                tc.tile_pool(name="pvps", bufs=1, space=bass.MemorySpace.PSUM) as pvps,
            ):
                def do_norm(pons, p, ih, on_act):
                    cols = slice(512 * ih, 512 * (ih + 1))
                    for s in (0, 1):
                        rec = rp.tile([1, 512], F32, tag="rec", name="rec")
                        if on_act:
                            lnd = rp.tile([1, 512], F32, tag="lnd", name="lnd")
                            nc.scalar.activation(lnd[:], pons[s][D : D + 1, :], LOG)
                            nc.scalar.activation(rec[:], lnd[:], EXP, scale=-1.0)
                        else:
                            nc.vector.reciprocal(rec[:], pons[s][D : D + 1, :])
                        rbc = rp.tile([64, 512], F32, tag="rbc", name="rbc")
                        nc.gpsimd.partition_broadcast(rbc[:], rec[:])
                        nc.vector.tensor_tensor(
                            attn[p][64 * s : 64 * (s + 1), cols],
                            pons[s][0:D, :],
                            rbc[:],
                            MULT,
                        )

                def pv_flush(E, c0, nch, po, p):
                    for ci in range(nch):
                        jt, s = divmod(c0 + ci, 2)
                        nc.tensor.matmul(
                            po[s][:],
                            VT[jt][:, 2 * p + s, :],
                            E[:, 512 * ci : 512 * (ci + 1)],
                            start=(jt == 0),
                            stop=(jt == NJT - 1),
                        )

                def seg_finish(po, p, ih, prev_norm):
                    # copy num+den out (frees the PV psum slots), then run
                    # the previous segment's normalization chain
                    pons = [
                        rp.tile([D + 1, 512], F32, tag="pon", name="pon", bufs=4)
                        for s in (0, 1)
                    ]
                    for s in (0, 1):
                        nc.vector.tensor_copy(pons[s][:], po[s][:])
                    if prev_norm is not None:
                        do_norm(*prev_norm, on_act=False)
                    return (pons, p, ih)

                # The PV-flush lag is carried ACROSS segment boundaries: the
                # last tile of segment s flushes inside segment s+1's first
                # iteration, so the PE/ACT pipeline never drains mid-stream.
                pending = None  # (E, c0, nch, po, p) awaiting PV flush
                finish = None  # (po, p, ih) of the segment pending closure
                prev_norm = None
                for seg in range(8):
                    p, ih = divmod(seg, 2)
                    Qh = [
                        Qs[p][64 * s : 64 * (s + 1), 512 * ih : 512 * (ih + 1)]
                        for s in (0, 1)
                    ]
                    Kh = [Ks[p][64 * s : 64 * (s + 1), :] for s in (0, 1)]
                    po = [
                        pvps.tile([D + 1, 512], F32, tag=f"pv{s}", name=f"po{s}")
                        for s in (0, 1)
                    ]
                    c0 = 0
                    while c0 < 2 * NJT:
                        nch = min(3, 2 * NJT - c0)
                        ps = qkps.tile([128, 512 * nch], F32, tag="qk", name="psqk")
                        for ci in range(nch):
                            jt, s = divmod(c0 + ci, 2)
                            nc.tensor.matmul(
                                ps[:, 512 * ci : 512 * (ci + 1)],
                                Kh[s][:, 128 * jt : 128 * (jt + 1)],
                                Qh[s][:],
                                start=True,
                                stop=True,
                            )
                        if pending is not None:
                            pv_flush(*pending)
                            if pending[1] + pending[2] == 2 * NJT:
                                prev_norm = seg_finish(*finish, prev_norm)
                            pending = None
                        E = ep.tile([128, 512 * nch], BF16, tag="e", name="E")
                        nc.scalar.activation(E[:], ps[:], EXP)
                        pending = (E, c0, nch, po, p)
                        if c0 + nch == 2 * NJT:
                            finish = (po, p, ih)
                        c0 += nch
                pv_flush(*pending)
                prev_norm = seg_finish(*finish, prev_norm)
                do_norm(*prev_norm, on_act=True)

            # ---- phase 3: output projection ----
            with tc.tile_pool(name="ops", bufs=2, space=bass.MemorySpace.PSUM) as ops:
                for m in range(2):
                    ps = ops.tile([128, LQ], F32, tag="o", name="pso")
                    for k in range(3):
                        for n in range(2):
                            nc.tensor.matmul(
                                ps[:, 512 * n : 512 * (n + 1)],
                                wo[k][:, 128 * m : 128 * (m + 1)],
                                attn[k][:, 512 * n : 512 * (n + 1)],
                                start=(k == 0),
                                stop=False,
                            )
                    for half in range(2):
                        hr = slice(64 * half, 64 * (half + 1))
                        for n in range(2):
                            nc.tensor.matmul(
                                ps[:, 512 * n : 512 * (n + 1)],
                                wo[3][hr, 128 * m : 128 * (m + 1)],
                                attn[3][hr, 512 * n : 512 * (n + 1)],
                                start=False,
                                stop=(half == 1),
                            )
                    osb = op.tile([128, LQ], BF16, tag="osb", name="osb")
                    for n in range(2):
                        cols = slice(512 * n, 512 * (n + 1))
                        if m == 0:
                            nc.scalar.add(osb[:, cols], ps[:, cols], bias[m][:])
                        else:
                            nc.vector.tensor_scalar_add(
                                osb[:, cols], ps[:, cols], bias[m][:]
                            )
                        nc.sync.dma_start(
                            out_d.ap()[128 * m : 128 * (m + 1), cols], osb[:, cols]
                        )

    nc.compile()
    return nc


def get_nc():
    if "nc" not in _cached:
        _cached["nc"] = build_nc()
    return _cached["nc"]


def make_in_maps(x, w_qkv, w_out, b_out):
    import ml_dtypes

    bf16 = ml_dtypes.bfloat16
    wqkvT = np.ascontiguousarray(w_qkv.T.astype(bf16))
    woutT = np.ascontiguousarray(w_out.T.astype(bf16))
    bias = np.ascontiguousarray(b_out.astype(np.float32).reshape(C, 1))
    in_maps = []
    for i in range(8):
        b, qh = i // 2, i % 2
        xb = np.ascontiguousarray(x[b].astype(bf16))
        xq = np.ascontiguousarray(xb[:, qh * LQ : (qh + 1) * LQ])
        in_maps.append(
            {"x": xb, "xq": xq, "wqkvT": wqkvT, "woutT": woutT, "bias": bias}
        )
    return in_maps


def assemble(results):
    out = np.empty((B, C, L), dtype=np.float32)
    for i in range(8):
        b, qh = i // 2, i % 2
        out[b][:, qh * LQ : (qh + 1) * LQ] = np.asarray(
            results[i]["out"], dtype=np.float32
        )
    return out


def kernel(x, w_qkv, w_out, b_out):
    x = np.asarray(x, dtype=np.float32)
    w_qkv = np.asarray(w_qkv, dtype=np.float32)
    w_out = np.asarray(w_out, dtype=np.float32)
    b_out = np.asarray(b_out, dtype=np.float32)
    assert x.shape == (B, C, L), x.shape
    nc = get_nc()
    in_maps = make_in_maps(x, w_qkv, w_out, b_out)
    res = run_bass_kernel_spmd(nc, in_maps, list(range(8)), trace=False)
    return assemble(res.results)


# revision 51
# speedup vs baseline: 1.1083x; 1.0037x over previous
"""Multi-head attention (b=4, c=256, l=2048, 8 heads x 64) on 8 TRN2 NeuronCores.

Sharding: core i handles batch b = i//2 and query half qh = i%2 (1024 queries),
computing all 8 heads over the full 2048-key context. Outputs are disjoint
[256, 1024] slabs -> host-side concat only, no collectives.

Per-core kernel (all matmuls bf16, 1 cycle/row; fp32 PSUM accumulate):
  1. Q = Wq @ xq (1024 cols), K = Wk @ x (2048), VT = (Wv @ x)^T computed
     directly as x^T-stationary matmuls, laid out [l-tile 128, 8 heads x 65]
     with a ones column per head (col 64) for the softmax denominator.
  2. Per head h, per key-tile jt (16 x 128 keys):
       simT[j, i] = K_h(jt)^T . Q_h          (PSUM [128, 1024])
       E = exp(simT / 8)                     (ScalarE, PSUM -> SBUF)
       PV += VT'[jt, h]^T . E                (PSUM [65, 1024], accum over jt)
     Row 64 of PV = softmax denominator; rows 0..64 = numerator.
  3. recip = 1/PV[64] (DVE; last pair via exp(-ln) on ScalarE), partition-
     broadcast on GpSimd, attn = num * recip.
  4. out = WoutT^T . attn + bias, DMA to DRAM as bf16 (host upcasts).

Engine budget per core (measured): the kernel is DUAL-bound - ScalarE exp
stream ~143 us busy (128 ACTIVATEs @ 1114 ns) and PE ~168 us busy (QK+PV
stream 2 matmul columns per score element at 2.4 GHz vs exp's 1 element at
1.2 GHz -> both have a ~109 us floor + overheads). Wider exp tiles shift
the bottleneck to PE (measured 221 us at [128,1536]); the balanced [128,
1024] tiling with PV software-pipelined one iteration behind QK/exp holds
ACT ~97% busy mid-stream. Phase-1 DMA lead-in ~20 us, tail+postamble ~14
us -> 201-203 us total.
"""

import sys

if "/opt/trn_rl_repo" not in sys.path:
    sys.path.insert(0, "/opt/trn_rl_repo")

import numpy as np

import concourse.bass as bass
import concourse.mybir as mybir
import concourse.tile as tile
from concourse import bacc
from concourse.bass_utils import run_bass_kernel_spmd

F32 = mybir.dt.float32
F32R = mybir.dt.float32r
BF16 = mybir.dt.bfloat16
EXP = mybir.ActivationFunctionType.Exp
LOG = mybir.ActivationFunctionType.Ln
MULT = mybir.AluOpType.mult

B, C, L = 4, 256, 2048
H, D = 8, 64
HID = H * D  # 512
LQ = L // 2  # 1024 queries per core
NJT = L // 128  # 16 key tiles
SCALE = D**-0.5

_cached = {}


def r(ap):
    return ap


def build_nc():
    nc = bacc.Bacc(
        "TRN2",
        target_bir_lowering=False,
        debug=False,
        enable_asserts=False,
        num_devices=8,
    )
    x_d = nc.dram_tensor("x", [C, L], BF16, kind="ExternalInput")
    xq_d = nc.dram_tensor("xq", [C, LQ], BF16, kind="ExternalInput")
    wq_d = nc.dram_tensor("wqkvT", [C, 3 * HID], BF16, kind="ExternalInput")
    wo_d = nc.dram_tensor("woutT", [HID, C], BF16, kind="ExternalInput")
    bias_d = nc.dram_tensor("bias", [C, 1], F32, kind="ExternalInput")
    out_d = nc.dram_tensor("out", [C, LQ], BF16, kind="ExternalOutput")

    with tile.TileContext(nc) as tc:
        with (
            tc.tile_pool(name="const", bufs=1) as cp,
            tc.tile_pool(name="epool", bufs=6) as ep,
            tc.tile_pool(name="rpool", bufs=2) as rp,
            tc.tile_pool(name="opool", bufs=2) as op,
        ):
            # ---- persistent SBUF tensors ----
            xb = [cp.tile([128, L], BF16, tag=f"xb{k}", name=f"xb{k}") for k in range(2)]
            xq = [cp.tile([128, LQ], BF16, tag=f"xq{k}", name=f"xq{k}") for k in range(2)]
            wq = [cp.tile([128, 3 * HID], BF16, tag=f"wq{k}", name=f"wq{k}") for k in range(2)]
            wo = [cp.tile([128, C], BF16, tag=f"wo{k}", name=f"wo{k}") for k in range(4)]
            bias = [cp.tile([128, 1], F32, tag=f"bias{k}", name=f"bias{k}") for k in range(2)]
            Qs = [cp.tile([128, LQ], BF16, tag=f"Q{m}", name=f"Q{m}") for m in range(4)]
            Ks = [cp.tile([128, L], BF16, tag=f"K{m}", name=f"K{m}") for m in range(4)]
            VT = [cp.tile([128, H, D + 1], BF16, tag=f"VT{t}", name=f"VT{t}") for t in range(NJT)]
            attn = [cp.tile([128, LQ], BF16, tag=f"attn{m}", name=f"attn{m}") for m in range(4)]
            acc = [cp.tile([128, LQ], F32, tag=f"acc{m}", name=f"acc{m}") for m in range(2)]
            dum = cp.tile([1, 16], F32, tag="dum", name="dum")
            dumo = cp.tile([1, 16], F32, tag="dumo", name="dumo")
            nc.gpsimd.memset(dum[:], 1.0)
            nc.scalar.activation(dumo[:], dum[:], LOG)
            nc.scalar.activation(dumo[:], dum[:], EXP)

            # ---- DMA inputs ----
            nc.sync.dma_start(wq[0][:, 0:512], wq_d.ap()[0:128, 0:512])
            nc.gpsimd.dma_start(wq[1][:, 0:512], wq_d.ap()[128:256, 0:512])
            nc.sync.dma_start(wq[0][:, 512:640], wq_d.ap()[0:128, 512:640])
            nc.gpsimd.dma_start(wq[1][:, 512:640], wq_d.ap()[128:256, 512:640])
            for k in range(2):
                rows = slice(128 * k, 128 * (k + 1))
                nc.scalar.dma_start(xq[k][:, 0:512], xq_d.ap()[rows, 0:512])
            for k in range(2):
                rows = slice(128 * k, 128 * (k + 1))
                nc.scalar.dma_start(xq[k][:, 512:1024], xq_d.ap()[rows, 512:1024])
            for k in range(2):
                rows = slice(128 * k, 128 * (k + 1))
                nc.sync.dma_start(xb[k][:, 0:1024], x_d.ap()[rows, 0:1024])
            for k in range(2):
                rows = slice(128 * k, 128 * (k + 1))
                nc.gpsimd.dma_start(xb[k][:, 1024:2048], x_d.ap()[rows, 1024:2048])
            nc.scalar.dma_start(wq[0][:, 640:1536], wq_d.ap()[0:128, 640:1536])
            nc.gpsimd.dma_start(wq[1][:, 640:1536], wq_d.ap()[128:256, 640:1536])
            for k in range(4):
                nc.sync.dma_start(wo[k][:], wo_d.ap()[128 * k : 128 * (k + 1), :])
            for k in range(2):
                rows = slice(128 * k, 128 * (k + 1))
                nc.gpsimd.dma_start(bias[k][:], bias_d.ap()[rows, :])

            # ---- phase 1: projections ----
            with (
                tc.tile_pool(name="pps", bufs=2, space=bass.MemorySpace.PSUM) as pps,
                tc.tile_pool(name="vps", bufs=2, space=bass.MemorySpace.PSUM) as vps,
            ):
                def q_proj(m):
                    ps = pps.tile([128, LQ], F32, tag="proj", name="ps")
                    for k in range(2):
                        for n in range(2):
                            nc.tensor.matmul(
                                ps[:, 512 * n : 512 * (n + 1)],
                                wq[k][:, 128 * m : 128 * (m + 1)],
                                xq[k][:, 512 * n : 512 * (n + 1)],
                                start=(k == 0),
                                stop=(k == 1),
                            )
                    if m == 0:
                        for n in range(2):
                            cols = slice(512 * n, 512 * (n + 1))
                            nc.vector.tensor_scalar_mul(
                                Qs[m][:, cols], ps[:, cols], SCALE
                            )
                    else:
                        nc.scalar.mul(Qs[m][:], ps[:], SCALE)

                def k_proj(m):
                    for lh in range(2):
                        ps = pps.tile([128, LQ], F32, tag="proj", name="ps")
                        for k in range(2):
                            for n in range(2):
                                nc.tensor.matmul(
                                    ps[:, 512 * n : 512 * (n + 1)],
                                    wq[k][:, HID + 128 * m : HID + 128 * (m + 1)],
                                    xb[k][:, 1024 * lh + 512 * n : 1024 * lh + 512 * (n + 1)],
                                    start=(k == 0),
                                    stop=(k == 1),
                                )
                        if m == 0:
                            for n in range(2):
                                nc.scalar.copy(
                                    Ks[m][
                                        :,
                                        1024 * lh + 512 * n : 1024 * lh + 512 * (n + 1),
                                    ],
                                    ps[:, 512 * n : 512 * (n + 1)],
                                )
                        else:
                            nc.vector.tensor_copy(
                                Ks[m][:, 1024 * lh : 1024 * (lh + 1)], ps[:]
                            )

                def vt_proj(t):
                    ps = vps.tile([128, HID], F32, tag="vproj", name="psv")
                    for k in range(2):
                        nc.tensor.matmul(
                            ps[:],
                            xb[k][:, 128 * t : 128 * (t + 1)],
                            wq[k][:, 2 * HID : 3 * HID],
                            start=(k == 0),
                            stop=(k == 1),
                        )
                    nc.vector.tensor_copy(
                        VT[t][:, :, 0:D], ps[:].rearrange("p (h c) -> p h c", h=H)
                    )
                    nc.gpsimd.memset(VT[t][:, :, D : D + 1], 1.0)

                q_proj(0)
                k_proj(0)
                vt_proj(0)
                vt_proj(1)
                q_proj(1)
                k_proj(1)
                vt_proj(2)
                vt_proj(3)
                q_proj(2)
                k_proj(2)
                q_proj(3)
                k_proj(3)
                for t in range(4, NJT):
                    vt_proj(t)

            # ---- phase 2: attention ----
            # Segments = (head-pair, 512-query-half). Scores go to [128,
            # 1536] psum tiles (3 chunks of 512 cols, both heads interleaved
            # - exp is elementwise so mixed-head tiles are fine). This
            # amortizes the ACTIVATE overhead (1540 ns / 3 chunks vs 1114/2)
            # while the PV accumulators shrink to [65, 512] = 1 bank each:
            # qk 2x3 banks + pv 2x1 = 8 banks exactly. PV lags one tile; the
            # normalization chain runs one SEGMENT behind so the PV slots
            # free after two fast copies.
            with (
                tc.tile_pool(name="qkps", bufs=2, space=bass.MemorySpace.PSUM) as qkps,
                tc.tile_pool(name="pvps", bufs=1, space=bass.MemorySpace.PSUM) as pvps,
            ):
                def do_norm(pons, p, ih, on_act):
                    cols = slice(512 * ih, 512 * (ih + 1))
                    for s in (0, 1):
                        rec = rp.tile([1, 512], F32, tag="rec", name="rec")
                        if on_act:
                            lnd = rp.tile([1, 512], F32, tag="lnd", name="lnd")
                            nc.scalar.activation(lnd[:], pons[s][D : D + 1, :], LOG)
                            nc.scalar.activation(rec[:], lnd[:], EXP, scale=-1.0)
                        else:
                            nc.vector.reciprocal(rec[:], pons[s][D : D + 1, :])
                        rbc = rp.tile([64, 512], F32, tag="rbc", name="rbc")
                        nc.gpsimd.partition_broadcast(rbc[:], rec[:])
                        nc.vector.tensor_tensor(
                            attn[p][64 * s : 64 * (s + 1), cols],
                            pons[s][0:D, :],
                            rbc[:],
                            MULT,
                        )

                def pv_flush(E, c0, nch, po, p):
                    for ci in range(nch):
                        jt, s = divmod(c0 + ci, 2)
                        nc.tensor.matmul(
                            po[s][:],
                            VT[jt][:, 2 * p + s, :],
                            E[:, 512 * ci : 512 * (ci + 1)],
                            start=(jt == 0),
                            stop=(jt == NJT - 1),
                        )

                def seg_finish(po, p, ih, prev_norm):
                    # copy num+den out (frees the PV psum slots), then run
                    # the previous segment's normalization chain
                    pons = [
                        rp.tile([D + 1, 512], F32, tag="pon", name="pon", bufs=4)
                        for s in (0, 1)
                    ]
                    for s in (0, 1):
                        nc.vector.tensor_copy(pons[s][:], po[s][:])
                    if prev_norm is not None:
                        do_norm(*prev_norm, on_act=False)
                    return (pons, p, ih)

                # The PV-flush lag is carried ACROSS segment boundaries: the
                # last tile of segment s flushes inside segment s+1's first
                # iteration, so the PE/ACT pipeline never drains mid-stream.
                pending = None  # (E, c0, nch, po, p) awaiting PV flush
                finish = None  # (po, p, ih) of the segment pending closure
                prev_norm = None
                for seg in range(8):
                    p, ih = divmod(seg, 2)
                    Qh = [
                        Qs[p][64 * s : 64 * (s + 1), 512 * ih : 512 * (ih + 1)]
                        for s in (0, 1)
                    ]
                    Kh = [Ks[p][64 * s : 64 * (s + 1), :] for s in (0, 1)]
                    po = [
                        pvps.tile([D + 1, 512], F32, tag=f"pv{s}", name=f"po{s}")
                        for s in (0, 1)
                    ]
                    c0 = 0
                    while c0 < 2 * NJT:
                        nch = min(3, 2 * NJT - c0)
                        ps = qkps.tile([128, 512 * nch], F32, tag="qk", name="psqk")
                        for ci in range(nch):
                            jt, s = divmod(c0 + ci, 2)
                            nc.tensor.matmul(
                                ps[:, 512 * ci : 512 * (ci + 1)],
                                Kh[s][:, 128 * jt : 128 * (jt + 1)],
                                Qh[s][:],
                                start=True,
                                stop=True,
                            )
                        if pending is not None:
                            pv_flush(*pending)
                            if pending[1] + pending[2] == 2 * NJT:
                                prev_norm = seg_finish(*finish, prev_norm)
                            pending = None
                        E = ep.tile([128, 512 * nch], BF16, tag="e", name="E")
                        nc.scalar.activation(E[:], ps[:], EXP)
                        pending = (E, c0, nch, po, p)
                        if c0 + nch == 2 * NJT:
                            finish = (po, p, ih)
                        c0 += nch
                pv_flush(*pending)
                prev_norm = seg_finish(*finish, prev_norm)
                do_norm(*prev_norm, on_act=True)

            # ---- phase 3: output projection ----
            with tc.tile_pool(name="ops", bufs=2, space=bass.MemorySpace.PSUM) as ops:
                for m in range(2):
                    ps = ops.tile([128, LQ], F32, tag="o", name="pso")
                    for k in range(3):
                        for n in range(2):
                            nc.tensor.matmul(
                                ps[:, 512 * n : 512 * (n + 1)],
                                wo[k][:, 128 * m : 128 * (m + 1)],
                                attn[k][:, 512 * n : 512 * (n + 1)],
                                start=(k == 0),
                                stop=False,
                            )
                    for half in range(2):
                        hr = slice(64 * half, 64 * (half + 1))
                        for n in range(2):
                            nc.tensor.matmul(
                                ps[:, 512 * n : 512 * (n + 1)],
                                wo[3][hr, 128 * m : 128 * (m + 1)],
                                attn[3][hr, 512 * n : 512 * (n + 1)],
                                start=False,
                                stop=(half == 1),
                            )
                    osb = op.tile([128, LQ], BF16, tag="osb", name="osb")
                    for n in range(2):
                        cols = slice(512 * n, 512 * (n + 1))
                        if m == 0:
                            nc.scalar.add(osb[:, cols], ps[:, cols], bias[m][:])
                        else:
                            nc.vector.tensor_scalar_add(
                                osb[:, cols], ps[:, cols], bias[m][:]
                            )
                        nc.sync.dma_start(
                            out_d.ap()[128 * m : 128 * (m + 1), cols], osb[:, cols]
                        )

    nc.compile()
    return nc


def get_nc():
    if "nc" not in _cached:
        _cached["nc"] = build_nc()
    return _cached["nc"]


def make_in_maps(x, w_qkv, w_out, b_out):
    import ml_dtypes

    bf16 = ml_dtypes.bfloat16
    wqkvT = np.ascontiguousarray(w_qkv.T.astype(bf16))
    woutT = np.ascontiguousarray(w_out.T.astype(bf16))
    bias = np.ascontiguousarray(b_out.astype(np.float32).reshape(C, 1))
    in_maps = []
    for i in range(8):
        b, qh = i // 2, i % 2
        xb = np.ascontiguousarray(x[b].astype(bf16))
        xq = np.ascontiguousarray(xb[:, qh * LQ : (qh + 1) * LQ])
        in_maps.append(
            {"x": xb, "xq": xq, "wqkvT": wqkvT, "woutT": woutT, "bias": bias}
        )
    return in_maps


def assemble(results):
    out = np.empty((B, C, L), dtype=np.float32)
    for i in range(8):
        b, qh = i // 2, i % 2
        out[b][:, qh * LQ : (qh + 1) * LQ] = np.asarray(
            results[i]["out"], dtype=np.float32
        )
    return out


def kernel(x, w_qkv, w_out, b_out):
    x = np.asarray(x, dtype=np.float32)
    w_qkv = np.asarray(w_qkv, dtype=np.float32)
    w_out = np.asarray(w_out, dtype=np.float32)
    b_out = np.asarray(b_out, dtype=np.float32)
    assert x.shape == (B, C, L), x.shape
    nc = get_nc()
    in_maps = make_in_maps(x, w_qkv, w_out, b_out)
    res = run_bass_kernel_spmd(nc, in_maps, list(range(8)), trace=False)
    return assemble(res.results)


# revision 52
# speedup vs baseline: 1.1116x; 1.0029x over previous
"""Multi-head attention (b=4, c=256, l=2048, 8 heads x 64) on 8 TRN2 NeuronCores.

Sharding: core i handles batch b = i//2 and query half qh = i%2 (1024 queries),
computing all 8 heads over the full 2048-key context. Outputs are disjoint
[256, 1024] slabs -> host-side concat only, no collectives.

Per-core kernel (bf16 matmuls at 1 cycle/row, fp32 PSUM accumulate):
  1. Q = Wq @ xq, K = Wk @ x, VT = (Wv @ x)^T projected directly into a
     transposed [l-tile 128, 8 heads x 65] layout whose ones column (64)
     makes the PV matmul emit the softmax denominator for free.
  2. Attention runs in 8 SEGMENTS = (head-pair, 512-query-half). Scores go
     to [128, 1536] psum tiles whose 3 x 512-column chunks interleave BOTH
     heads of the pair (exp is elementwise, mixed-head tiles are fine) --
     amortizing the ~310-cycle ACTIVATE overhead. PSUM: 2 x 3-bank score
     slots (double-buffered) + 2 x 1-bank [65, 512] PV accumulators = 8.
  3. Three nested software pipelines with no drain points: PV flushes lag
     the QK/exp stream by one tile, the normalization chain (psum-freeing
     copy, reciprocal, GpSimd partition-broadcast, numerator multiply) lags
     by one segment, and BOTH lags are carried ACROSS segment boundaries so
     the 134 us exp stream runs with zero gaps. The last segment's
     reciprocal uses exp(-ln) on the then-idle ScalarE.
  4. out = WoutT^T . attn + bias, DMA to DRAM as bf16 (host upcasts).

Measured: ScalarE and TensorE both ~147 us busy (dual roofline: QK+PV
stream 2 matmul columns per score element at 2.4 GHz vs exp's 1 element per
lane at 1.2 GHz -> identical ~109 us floors). Span ~197 us = ~42 us lead-in
(NEFF preamble + 8-core DMA contention + projection FIFO, deadline-pinned)
+ 134 us gapless exp stream + ~18 us tail incl ~7 us fixed postamble.
"""

import sys

if "/opt/trn_rl_repo" not in sys.path:
    sys.path.insert(0, "/opt/trn_rl_repo")

import numpy as np

import concourse.bass as bass
import concourse.mybir as mybir
import concourse.tile as tile
from concourse import bacc
from concourse.bass_utils import run_bass_kernel_spmd

F32 = mybir.dt.float32
F32R = mybir.dt.float32r
BF16 = mybir.dt.bfloat16
EXP = mybir.ActivationFunctionType.Exp
LOG = mybir.ActivationFunctionType.Ln
MULT = mybir.AluOpType.mult

B, C, L = 4, 256, 2048
H, D = 8, 64
HID = H * D  # 512
LQ = L // 2  # 1024 queries per core
NJT = L // 128  # 16 key tiles
SCALE = D**-0.5

_cached = {}


def r(ap):
    return ap


def build_nc():
    nc = bacc.Bacc(
        "TRN2",
        target_bir_lowering=False,
        debug=False,
        enable_asserts=False,
        num_devices=8,
    )
    x_d = nc.dram_tensor("x", [C, L], BF16, kind="ExternalInput")
    xq_d = nc.dram_tensor("xq", [C, LQ], BF16, kind="ExternalInput")
    wq_d = nc.dram_tensor("wqkvT", [C, 3 * HID], BF16, kind="ExternalInput")
    wo_d = nc.dram_tensor("woutT", [HID, C], BF16, kind="ExternalInput")
    bias_d = nc.dram_tensor("bias", [C, 1], F32, kind="ExternalInput")
    out_d = nc.dram_tensor("out", [C, LQ], BF16, kind="ExternalOutput")

    with tile.TileContext(nc) as tc:
        with (
            tc.tile_pool(name="const", bufs=1) as cp,
            tc.tile_pool(name="epool", bufs=6) as ep,
            tc.tile_pool(name="rpool", bufs=2) as rp,
            tc.tile_pool(name="opool", bufs=2) as op,
        ):
            # ---- persistent SBUF tensors ----
            xb = [cp.tile([128, L], BF16, tag=f"xb{k}", name=f"xb{k}") for k in range(2)]
            xq = [cp.tile([128, LQ], BF16, tag=f"xq{k}", name=f"xq{k}") for k in range(2)]
            wq = [cp.tile([128, 3 * HID], BF16, tag=f"wq{k}", name=f"wq{k}") for k in range(2)]
            wo = [cp.tile([128, C], BF16, tag=f"wo{k}", name=f"wo{k}") for k in range(4)]
            bias = [cp.tile([128, 1], F32, tag=f"bias{k}", name=f"bias{k}") for k in range(2)]
            Qs = [cp.tile([128, LQ], BF16, tag=f"Q{m}", name=f"Q{m}") for m in range(4)]
            Ks = [cp.tile([128, L], BF16, tag=f"K{m}", name=f"K{m}") for m in range(4)]
            VT = [cp.tile([128, H, D + 1], BF16, tag=f"VT{t}", name=f"VT{t}") for t in range(NJT)]
            attn = [cp.tile([128, LQ], BF16, tag=f"attn{m}", name=f"attn{m}") for m in range(4)]
            acc = [cp.tile([128, LQ], F32, tag=f"acc{m}", name=f"acc{m}") for m in range(2)]
            dum = cp.tile([1, 16], F32, tag="dum", name="dum")
            dumo = cp.tile([1, 16], F32, tag="dumo", name="dumo")
            nc.gpsimd.memset(dum[:], 1.0)
            nc.scalar.activation(dumo[:], dum[:], LOG)
            nc.scalar.activation(dumo[:], dum[:], EXP)

            # ---- DMA inputs ----
            nc.sync.dma_start(wq[0][:, 0:512], wq_d.ap()[0:128, 0:512])
            nc.gpsimd.dma_start(wq[1][:, 0:512], wq_d.ap()[128:256, 0:512])
            nc.sync.dma_start(wq[0][:, 512:640], wq_d.ap()[0:128, 512:640])
            nc.gpsimd.dma_start(wq[1][:, 512:640], wq_d.ap()[128:256, 512:640])
            for k in range(2):
                rows = slice(128 * k, 128 * (k + 1))
                nc.scalar.dma_start(xq[k][:, 0:512], xq_d.ap()[rows, 0:512])
            for k in range(2):
                rows = slice(128 * k, 128 * (k + 1))
                nc.scalar.dma_start(xq[k][:, 512:1024], xq_d.ap()[rows, 512:1024])
            for k in range(2):
                rows = slice(128 * k, 128 * (k + 1))
                nc.sync.dma_start(xb[k][:, 0:1024], x_d.ap()[rows, 0:1024])
            for k in range(2):
                rows = slice(128 * k, 128 * (k + 1))
                nc.gpsimd.dma_start(xb[k][:, 1024:2048], x_d.ap()[rows, 1024:2048])
            nc.scalar.dma_start(wq[0][:, 640:1536], wq_d.ap()[0:128, 640:1536])
            nc.gpsimd.dma_start(wq[1][:, 640:1536], wq_d.ap()[128:256, 640:1536])
            for k in range(4):
                nc.sync.dma_start(wo[k][:], wo_d.ap()[128 * k : 128 * (k + 1), :])
            for k in range(2):
                rows = slice(128 * k, 128 * (k + 1))
                nc.gpsimd.dma_start(bias[k][:], bias_d.ap()[rows, :])

            # ---- phase 1: projections ----
            with (
                tc.tile_pool(name="pps", bufs=2, space=bass.MemorySpace.PSUM) as pps,
                tc.tile_pool(name="vps", bufs=2, space=bass.MemorySpace.PSUM) as vps,
            ):
                def q_proj(m):
                    ps = pps.tile([128, LQ], F32, tag="proj", name="ps")
                    for k in range(2):
                        for n in range(2):
                            nc.tensor.matmul(
                                ps[:, 512 * n : 512 * (n + 1)],
                                wq[k][:, 128 * m : 128 * (m + 1)],
                                xq[k][:, 512 * n : 512 * (n + 1)],
                                start=(k == 0),
                                stop=(k == 1),
                            )
                    if m == 0:
                        for n in range(2):
                            cols = slice(512 * n, 512 * (n + 1))
                            nc.vector.tensor_scalar_mul(
                                Qs[m][:, cols], ps[:, cols], SCALE
                            )
                    else:
                        nc.scalar.mul(Qs[m][:], ps[:], SCALE)

                def k_proj(m):
                    for lh in range(2):
                        ps = pps.tile([128, LQ], F32, tag="proj", name="ps")
                        for k in range(2):
                            for n in range(2):
                                nc.tensor.matmul(
                                    ps[:, 512 * n : 512 * (n + 1)],
                                    wq[k][:, HID + 128 * m : HID + 128 * (m + 1)],
                                    xb[k][:, 1024 * lh + 512 * n : 1024 * lh + 512 * (n + 1)],
                                    start=(k == 0),
                                    stop=(k == 1),
                                )
                        if m == 0:
                            for n in range(2):
                                nc.scalar.copy(
                                    Ks[m][
                                        :,
                                        1024 * lh + 512 * n : 1024 * lh + 512 * (n + 1),
                                    ],
                                    ps[:, 512 * n : 512 * (n + 1)],
                                )
                        else:
                            nc.vector.tensor_copy(
                                Ks[m][:, 1024 * lh : 1024 * (lh + 1)], ps[:]
                            )

                def vt_proj(t):
                    ps = vps.tile([128, HID], F32, tag="vproj", name="psv")
                    for k in range(2):
                        nc.tensor.matmul(
                            ps[:],
                            xb[k][:, 128 * t : 128 * (t + 1)],
                            wq[k][:, 2 * HID : 3 * HID],
                            start=(k == 0),
                            stop=(k == 1),
                        )
                    nc.vector.tensor_copy(
                        VT[t][:, :, 0:D], ps[:].rearrange("p (h c) -> p h c", h=H)
                    )
                    nc.gpsimd.memset(VT[t][:, :, D : D + 1], 1.0)

                q_proj(0)
                k_proj(0)
                vt_proj(0)
                vt_proj(1)
                q_proj(1)
                k_proj(1)
                vt_proj(2)
                vt_proj(3)
                q_proj(2)
                k_proj(2)
                q_proj(3)
                k_proj(3)
                for t in range(4, NJT):
                    vt_proj(t)

            # ---- phase 2: attention ----
            # Segments = (head-pair, 512-query-half). Scores go to [128,
            # 1536] psum tiles (3 chunks of 512 cols, both heads interleaved
            # - exp is elementwise so mixed-head tiles are fine). This
            # amortizes the ACTIVATE overhead (1540 ns / 3 chunks vs 1114/2)
            # while the PV accumulators shrink to [65, 512] = 1 bank each:
            # qk 2x3 banks + pv 2x1 = 8 banks exactly. PV lags one tile; the
            # normalization chain runs one SEGMENT behind so the PV slots
            # free after two fast copies.
            with (
                tc.tile_pool(name="qkps", bufs=2, space=bass.MemorySpace.PSUM) as qkps,
                tc.tile_pool(name="pvps", bufs=1, space=bass.MemorySpace.PSUM) as pvps,
            ):
                def do_norm(pons, p, ih, on_act):
                    cols = slice(512 * ih, 512 * (ih + 1))
                    for s in (0, 1):
                        rec = rp.tile([1, 512], F32, tag="rec", name="rec")
                        if on_act:
                            lnd = rp.tile([1, 512], F32, tag="lnd", name="lnd")
                            nc.scalar.activation(lnd[:], pons[s][D : D + 1, :], LOG)
                            nc.scalar.activation(rec[:], lnd[:], EXP, scale=-1.0)
                        else:
                            nc.vector.reciprocal(rec[:], pons[s][D : D + 1, :])
                        rbc = rp.tile([64, 512], F32, tag="rbc", name="rbc")
                        nc.gpsimd.partition_broadcast(rbc[:], rec[:])
                        nc.vector.tensor_tensor(
                            attn[p][64 * s : 64 * (s + 1), cols],
                            pons[s][0:D, :],
                            rbc[:],
                            MULT,
                        )

                def pv_flush(E, c0, nch, po, p):
                    for ci in range(nch):
                        jt, s = divmod(c0 + ci, 2)
                        nc.tensor.matmul(
                            po[s][:],
                            VT[jt][:, 2 * p + s, :],
                            E[:, 512 * ci : 512 * (ci + 1)],
                            start=(jt == 0),
                            stop=(jt == NJT - 1),
                        )

                def seg_finish(po, p, ih, prev_norm):
                    # copy num+den out (frees the PV psum slots), then run
                    # the previous segment's normalization chain
                    pons = [
                        rp.tile([D + 1, 512], F32, tag="pon", name="pon", bufs=4)
                        for s in (0, 1)
                    ]
                    for s in (0, 1):
                        nc.vector.tensor_copy(pons[s][:], po[s][:])
                    if prev_norm is not None:
                        do_norm(*prev_norm, on_act=False)
                    return (pons, p, ih)

                # The PV-flush lag is carried ACROSS segment boundaries: the
                # last tile of segment s flushes inside segment s+1's first
                # iteration, so the PE/ACT pipeline never drains mid-stream.
                pending = None  # (E, c0, nch, po, p) awaiting PV flush
                finish = None  # (po, p, ih) of the segment pending closure
                prev_norm = None
                for seg in range(8):
                    p, ih = divmod(seg, 2)
                    Qh = [
                        Qs[p][64 * s : 64 * (s + 1), 512 * ih : 512 * (ih + 1)]
                        for s in (0, 1)
                    ]
                    Kh = [Ks[p][64 * s : 64 * (s + 1), :] for s in (0, 1)]
                    po = [
                        pvps.tile([D + 1, 512], F32, tag=f"pv{s}", name=f"po{s}")
                        for s in (0, 1)
                    ]
                    c0 = 0
                    while c0 < 2 * NJT:
                        nch = min(3, 2 * NJT - c0)
                        ps = qkps.tile([128, 512 * nch], F32, tag="qk", name="psqk")
                        for ci in range(nch):
                            jt, s = divmod(c0 + ci, 2)
                            nc.tensor.matmul(
                                ps[:, 512 * ci : 512 * (ci + 1)],
                                Kh[s][:, 128 * jt : 128 * (jt + 1)],
                                Qh[s][:],
                                start=True,
                                stop=True,
                            )
                        if pending is not None:
                            pv_flush(*pending)
                            if pending[1] + pending[2] == 2 * NJT:
                                prev_norm = seg_finish(*finish, prev_norm)
                            pending = None
                        E = ep.tile([128, 512 * nch], BF16, tag="e", name="E")
                        nc.scalar.activation(E[:], ps[:], EXP)
                        pending = (E, c0, nch, po, p)
                        if c0 + nch == 2 * NJT:
                            finish = (po, p, ih)
                        c0 += nch
                pv_flush(*pending)
                prev_norm = seg_finish(*finish, prev_norm)
                do_norm(*prev_norm, on_act=True)

            # ---- phase 3: output projection ----
            with tc.tile_pool(name="ops", bufs=2, space=bass.MemorySpace.PSUM) as ops:
                for m in range(2):
                    ps = ops.tile([128, LQ], F32, tag="o", name="pso")
                    for k in range(3):
                        for n in range(2):
                            nc.tensor.matmul(
                                ps[:, 512 * n : 512 * (n + 1)],
                                wo[k][:, 128 * m : 128 * (m + 1)],
                                attn[k][:, 512 * n : 512 * (n + 1)],
                                start=(k == 0),
                                stop=False,
                            )
                    for half in range(2):
                        hr = slice(64 * half, 64 * (half + 1))
                        for n in range(2):
                            nc.tensor.matmul(
                                ps[:, 512 * n : 512 * (n + 1)],
                                wo[3][hr, 128 * m : 128 * (m + 1)],
                                attn[3][hr, 512 * n : 512 * (n + 1)],
                                start=False,
                                stop=(half == 1),
                            )
                    osb = op.tile([128, LQ], BF16, tag="osb", name="osb")
                    for n in range(2):
                        cols = slice(512 * n, 512 * (n + 1))
                        if m == 0:
                            nc.scalar.add(osb[:, cols], ps[:, cols], bias[m][:])
                        else:
                            nc.vector.tensor_scalar_add(
                                osb[:, cols], ps[:, cols], bias[m][:]
                            )
                        nc.sync.dma_start(
                            out_d.ap()[128 * m : 128 * (m + 1), cols], osb[:, cols]
                        )

    nc.compile()
    return nc


def get_nc():
    if "nc" not in _cached:
        _cached["nc"] = build_nc()
    return _cached["nc"]


def make_in_maps(x, w_qkv, w_out, b_out):
    import ml_dtypes

    bf16 = ml_dtypes.bfloat16
    wqkvT = np.ascontiguousarray(w_qkv.T.astype(bf16))
    woutT = np.ascontiguousarray(w_out.T.astype(bf16))
    bias = np.ascontiguousarray(b_out.astype(np.float32).reshape(C, 1))
    in_maps = []
    for i in range(8):
        b, qh = i // 2, i % 2
        xb = np.ascontiguousarray(x[b].astype(bf16))
        xq = np.ascontiguousarray(xb[:, qh * LQ : (qh + 1) * LQ])
        in_maps.append(
            {"x": xb, "xq": xq, "wqkvT": wqkvT, "woutT": woutT, "bias": bias}
        )
    return in_maps


def assemble(results):
    out = np.empty((B, C, L), dtype=np.float32)
    for i in range(8):
        b, qh = i // 2, i % 2
        out[b][:, qh * LQ : (qh + 1) * LQ] = np.asarray(
            results[i]["out"], dtype=np.float32
        )
    return out


def kernel(x, w_qkv, w_out, b_out):
    x = np.asarray(x, dtype=np.float32)
    w_qkv = np.asarray(w_qkv, dtype=np.float32)
    w_out = np.asarray(w_out, dtype=np.float32)
    b_out = np.asarray(b_out, dtype=np.float32)
    assert x.shape == (B, C, L), x.shape
    nc = get_nc()
    in_maps = make_in_maps(x, w_qkv, w_out, b_out)
    res = run_bass_kernel_spmd(nc, in_maps, list(range(8)), trace=False)
    return assemble(res.results)


# revision 53
# speedup vs baseline: 1.1746x; 1.0567x over previous
"""Multi-head attention (b=4, c=256, l=2048, 8 heads x 64) on 8 TRN2 NeuronCores.

Sharding: core i handles batch b = i//2 and query half qh = i%2 (1024 queries),
computing all 8 heads over the full 2048-key context. Outputs are disjoint
[256, 1024] slabs -> host-side concat only, no collectives.

Per-core kernel (bf16 matmuls at 1 cycle/row, fp32 PSUM accumulate):
  1. Q = Wq @ xq, K = Wk @ x, VT = (Wv @ x)^T projected directly into a
     transposed [l-tile 128, 8 heads x 65] layout whose ones column (64)
     makes the PV matmul emit the softmax denominator for free.
  2. Attention runs in 8 SEGMENTS = (head-pair, 512-query-half). Scores go
     to [128, 1536] psum tiles whose 3 x 512-column chunks interleave BOTH
     heads of the pair (exp is elementwise, mixed-head tiles are fine) --
     amortizing the ~310-cycle ACTIVATE overhead. PSUM: 2 x 3-bank score
     slots (double-buffered) + 2 x 1-bank [65, 512] PV accumulators = 8.
  3. Three nested software pipelines with no drain points: PV flushes lag
     the QK/exp stream by one tile, the normalization chain (psum-freeing
     copy, reciprocal, GpSimd partition-broadcast, numerator multiply) lags
     by one segment, and BOTH lags are carried ACROSS segment boundaries so
     the 134 us exp stream runs with zero gaps. The last segment's
     reciprocal uses exp(-ln) on the then-idle ScalarE.
  4. out = WoutT^T . attn + bias, DMA to DRAM as bf16 (host upcasts).

Measured: ScalarE and TensorE both ~147 us busy (dual roofline: QK+PV
stream 2 matmul columns per score element at 2.4 GHz vs exp's 1 element per
lane at 1.2 GHz -> identical ~109 us floors). Span ~197 us = ~42 us lead-in
(NEFF preamble + 8-core DMA contention + projection FIFO, deadline-pinned)
+ 134 us gapless exp stream + ~18 us tail incl ~7 us fixed postamble.
"""

import sys

if "/opt/trn_rl_repo" not in sys.path:
    sys.path.insert(0, "/opt/trn_rl_repo")

import numpy as np

import concourse.bass as bass
import concourse.mybir as mybir
import concourse.tile as tile
from concourse import bacc
from concourse.bass_utils import run_bass_kernel_spmd

F32 = mybir.dt.float32
F32R = mybir.dt.float32r
BF16 = mybir.dt.bfloat16
EXP = mybir.ActivationFunctionType.Exp
LOG = mybir.ActivationFunctionType.Ln
MULT = mybir.AluOpType.mult

B, C, L = 4, 256, 2048
H, D = 8, 64
HID = H * D  # 512
LQ = L // 2  # 1024 queries per core
NJT = L // 128  # 16 key tiles
SCALE = D**-0.5

_cached = {}


def r(ap):
    return ap


def build_nc():
    nc = bacc.Bacc(
        "TRN2",
        target_bir_lowering=False,
        debug=False,
        enable_asserts=False,
        num_devices=8,
    )
    x_d = nc.dram_tensor("x", [C, L], BF16, kind="ExternalInput")
    xq_d = nc.dram_tensor("xq", [C, LQ], BF16, kind="ExternalInput")
    wq_d = nc.dram_tensor("wqkvT", [C, 3 * HID], BF16, kind="ExternalInput")
    wo_d = nc.dram_tensor("woutT", [HID, C], BF16, kind="ExternalInput")
    bias_d = nc.dram_tensor("bias", [C, 1], F32, kind="ExternalInput")
    out_d = nc.dram_tensor("out", [C, LQ], BF16, kind="ExternalOutput")

    with tile.TileContext(nc) as tc:
        with (
            tc.tile_pool(name="const", bufs=1) as cp,
            tc.tile_pool(name="epool", bufs=6) as ep,
            tc.tile_pool(name="rpool", bufs=2) as rp,
            tc.tile_pool(name="opool", bufs=2) as op,
        ):
            # ---- persistent SBUF tensors ----
            xb = [cp.tile([128, L], BF16, tag=f"xb{k}", name=f"xb{k}") for k in range(2)]
            xq = [cp.tile([128, LQ], BF16, tag=f"xq{k}", name=f"xq{k}") for k in range(2)]
            wq = [cp.tile([128, 3 * HID], BF16, tag=f"wq{k}", name=f"wq{k}") for k in range(2)]
            wo = [cp.tile([128, C], BF16, tag=f"wo{k}", name=f"wo{k}") for k in range(4)]
            bias = [cp.tile([128, 1], F32, tag=f"bias{k}", name=f"bias{k}") for k in range(2)]
            Qs = [cp.tile([128, LQ], BF16, tag=f"Q{m}", name=f"Q{m}") for m in range(4)]
            Ks = [cp.tile([128, L], BF16, tag=f"K{m}", name=f"K{m}") for m in range(4)]
            VT = [cp.tile([128, H, D + 1], BF16, tag=f"VT{t}", name=f"VT{t}") for t in range(NJT)]
            attn = [cp.tile([128, LQ], BF16, tag=f"attn{m}", name=f"attn{m}") for m in range(4)]
            acc = [cp.tile([128, LQ], F32, tag=f"acc{m}", name=f"acc{m}") for m in range(2)]
            dum = cp.tile([1, 16], F32, tag="dum", name="dum")
            dumo = cp.tile([1, 16], F32, tag="dumo", name="dumo")
            nc.gpsimd.memset(dum[:], 1.0)
            nc.scalar.activation(dumo[:], dum[:], LOG)
            nc.scalar.activation(dumo[:], dum[:], EXP)

            # ---- DMA inputs ----
            nc.sync.dma_start(wq[0][:, 0:512], wq_d.ap()[0:128, 0:512])
            nc.gpsimd.dma_start(wq[1][:, 0:512], wq_d.ap()[128:256, 0:512])
            nc.sync.dma_start(wq[0][:, 512:640], wq_d.ap()[0:128, 512:640])
            nc.gpsimd.dma_start(wq[1][:, 512:640], wq_d.ap()[128:256, 512:640])
            for k in range(2):
                rows = slice(128 * k, 128 * (k + 1))
                nc.scalar.dma_start(xq[k][:, 0:512], xq_d.ap()[rows, 0:512])
            for k in range(2):
                rows = slice(128 * k, 128 * (k + 1))
                nc.scalar.dma_start(xq[k][:, 512:1024], xq_d.ap()[rows, 512:1024])
            for k in range(2):
                rows = slice(128 * k, 128 * (k + 1))
                nc.sync.dma_start(xb[k][:, 0:1024], x_d.ap()[rows, 0:1024])
            for k in range(2):
                rows = slice(128 * k, 128 * (k + 1))
                nc.gpsimd.dma_start(xb[k][:, 1024:2048], x_d.ap()[rows, 1024:2048])
            nc.scalar.dma_start(wq[0][:, 640:1536], wq_d.ap()[0:128, 640:1536])
            nc.gpsimd.dma_start(wq[1][:, 640:1536], wq_d.ap()[128:256, 640:1536])
            for k in range(4):
                nc.sync.dma_start(wo[k][:], wo_d.ap()[128 * k : 128 * (k + 1), :])
            for k in range(2):
                rows = slice(128 * k, 128 * (k + 1))
                nc.gpsimd.dma_start(bias[k][:], bias_d.ap()[rows, :])

            # ---- phase 1: projections ----
            with (
                tc.tile_pool(name="pps", bufs=2, space=bass.MemorySpace.PSUM) as pps,
                tc.tile_pool(name="vps", bufs=2, space=bass.MemorySpace.PSUM) as vps,
            ):
                def q_proj(m):
                    ps = pps.tile([128, LQ], F32, tag="proj", name="ps")
                    for k in range(2):
                        for n in range(2):
                            nc.tensor.matmul(
                                ps[:, 512 * n : 512 * (n + 1)],
                                wq[k][:, 128 * m : 128 * (m + 1)],
                                xq[k][:, 512 * n : 512 * (n + 1)],
                                start=(k == 0),
                                stop=(k == 1),
                            )
                    if m == 0:
                        for n in range(2):
                            cols = slice(512 * n, 512 * (n + 1))
                            nc.vector.tensor_scalar_mul(
                                Qs[m][:, cols], ps[:, cols], SCALE
                            )
                    else:
                        nc.scalar.mul(Qs[m][:], ps[:], SCALE)

                def k_proj(m):
                    for lh in range(2):
                        ps = pps.tile([128, LQ], F32, tag="proj", name="ps")
                        for k in range(2):
                            for n in range(2):
                                nc.tensor.matmul(
                                    ps[:, 512 * n : 512 * (n + 1)],
                                    wq[k][:, HID + 128 * m : HID + 128 * (m + 1)],
                                    xb[k][:, 1024 * lh + 512 * n : 1024 * lh + 512 * (n + 1)],
                                    start=(k == 0),
                                    stop=(k == 1),
                                )
                        if m == 0:
                            for n in range(2):
                                nc.scalar.copy(
                                    Ks[m][
                                        :,
                                        1024 * lh + 512 * n : 1024 * lh + 512 * (n + 1),
                                    ],
                                    ps[:, 512 * n : 512 * (n + 1)],
                                )
                        else:
                            nc.vector.tensor_copy(
                                Ks[m][:, 1024 * lh : 1024 * (lh + 1)], ps[:]
                            )

                def vt_proj(t):
                    ps = vps.tile([128, HID], F32, tag="vproj", name="psv")
                    for k in range(2):
                        nc.tensor.matmul(
                            ps[:],
                            xb[k][:, 128 * t : 128 * (t + 1)],
                            wq[k][:, 2 * HID : 3 * HID],
                            start=(k == 0),
                            stop=(k == 1),
                        )
                    nc.vector.tensor_copy(
                        VT[t][:, :, 0:D], ps[:].rearrange("p (h c) -> p h c", h=H)
                    )
                    nc.gpsimd.memset(VT[t][:, :, D : D + 1], 1.0)

                q_proj(0)
                k_proj(0)
                vt_proj(0)
                vt_proj(1)
                # hoist the first 3 score tiles of segment 0 into the
                # phase-1 window: QK+exp run here (filling the otherwise
                # idle ScalarE during the DMA lead-in); their PV flushes
                # drain through the main loop's pending FIFO.
                hoist = []
                for hc in range(3):
                    ps = pps.tile([128, 1024], F32, tag="proj", name="hqk")
                    for ci in range(2):
                        jt, s = divmod(2 * hc + ci, 2)
                        nc.tensor.matmul(
                            ps[:, 512 * ci : 512 * (ci + 1)],
                            Ks[0][64 * s : 64 * (s + 1), 128 * jt : 128 * (jt + 1)],
                            Qs[0][64 * s : 64 * (s + 1), 0:512],
                            start=True,
                            stop=True,
                        )
                    Eh = ep.tile([128, 1024], BF16, tag="e", name="Eh")
                    nc.scalar.activation(Eh[:], ps[:], EXP)
                    hoist.append((Eh, 2 * hc, 2))
                q_proj(1)
                k_proj(1)
                vt_proj(2)
                vt_proj(3)
                q_proj(2)
                k_proj(2)
                q_proj(3)
                k_proj(3)
                for t in range(4, NJT):
                    vt_proj(t)

            # ---- phase 2: attention ----
            # Segments = (head-pair, 512-query-half). Scores go to [128,
            # 1536] psum tiles (3 chunks of 512 cols, both heads interleaved
            # - exp is elementwise so mixed-head tiles are fine). This
            # amortizes the ACTIVATE overhead (1540 ns / 3 chunks vs 1114/2)
            # while the PV accumulators shrink to [65, 512] = 1 bank each:
            # qk 2x3 banks + pv 2x1 = 8 banks exactly. PV lags one tile; the
            # normalization chain runs one SEGMENT behind so the PV slots
            # free after two fast copies.
            with (
                tc.tile_pool(name="qkps", bufs=2, space=bass.MemorySpace.PSUM) as qkps,
                tc.tile_pool(name="pvps", bufs=1, space=bass.MemorySpace.PSUM) as pvps,
            ):
                def do_norm(pons, p, ih, on_act):
                    cols = slice(512 * ih, 512 * (ih + 1))
                    for s in (0, 1):
                        rec = rp.tile([1, 512], F32, tag="rec", name="rec")
                        if on_act:
                            lnd = rp.tile([1, 512], F32, tag="lnd", name="lnd")
                            nc.scalar.activation(lnd[:], pons[s][D : D + 1, :], LOG)
                            nc.scalar.activation(rec[:], lnd[:], EXP, scale=-1.0)
                        else:
                            nc.vector.reciprocal(rec[:], pons[s][D : D + 1, :])
                        rbc = rp.tile([64, 512], F32, tag="rbc", name="rbc")
                        nc.gpsimd.partition_broadcast(rbc[:], rec[:])
                        nc.vector.tensor_tensor(
                            attn[p][64 * s : 64 * (s + 1), cols],
                            pons[s][0:D, :],
                            rbc[:],
                            MULT,
                        )

                def pv_flush(E, c0, nch, po, p):
                    for ci in range(nch):
                        jt, s = divmod(c0 + ci, 2)
                        nc.tensor.matmul(
                            po[s][:],
                            VT[jt][:, 2 * p + s, :],
                            E[:, 512 * ci : 512 * (ci + 1)],
                            start=(jt == 0),
                            stop=(jt == NJT - 1),
                        )

                def seg_finish(po, p, ih, prev_norm):
                    # copy num+den out (frees the PV psum slots), then run
                    # the previous segment's normalization chain
                    pons = [
                        rp.tile([D + 1, 512], F32, tag="pon", name="pon", bufs=4)
                        for s in (0, 1)
                    ]
                    for s in (0, 1):
                        nc.vector.tensor_copy(pons[s][:], po[s][:])
                    if prev_norm is not None:
                        do_norm(*prev_norm, on_act=False)
                    return (pons, p, ih)

                # The PV-flush lag is carried ACROSS segment boundaries: the
                # last tile of segment s flushes inside segment s+1's first
                # iteration, so the PE/ACT pipeline never drains mid-stream.
                pending = []  # FIFO of (E, c0, nch, po, p, ih)
                prev_norm = None
                for seg in range(8):
                    p, ih = divmod(seg, 2)
                    Qh = [
                        Qs[p][64 * s : 64 * (s + 1), 512 * ih : 512 * (ih + 1)]
                        for s in (0, 1)
                    ]
                    Kh = [Ks[p][64 * s : 64 * (s + 1), :] for s in (0, 1)]
                    po = [
                        pvps.tile([D + 1, 512], F32, tag=f"pv{s}", name=f"po{s}")
                        for s in (0, 1)
                    ]
                    c0 = 0
                    if seg == 0:
                        pending = [(E, hc0, hn, po, p, ih) for (E, hc0, hn) in hoist]
                        c0 = 6
                    while c0 < 2 * NJT:
                        nch = min(3, 2 * NJT - c0)
                        ps = qkps.tile([128, 512 * nch], F32, tag="qk", name="psqk")
                        for ci in range(nch):
                            jt, s = divmod(c0 + ci, 2)
                            nc.tensor.matmul(
                                ps[:, 512 * ci : 512 * (ci + 1)],
                                Kh[s][:, 128 * jt : 128 * (jt + 1)],
                                Qh[s][:],
                                start=True,
                                stop=True,
                            )
                        if pending:
                            ent = pending.pop(0)
                            pv_flush(*ent[:5])
                            if ent[1] + ent[2] == 2 * NJT:
                                prev_norm = seg_finish(ent[3], ent[4], ent[5], prev_norm)
                        E = ep.tile([128, 512 * nch], BF16, tag="e", name="E")
                        nc.scalar.activation(E[:], ps[:], EXP)
                        pending.append((E, c0, nch, po, p, ih))
                        c0 += nch
                for ent in pending:
                    pv_flush(*ent[:5])
                    if ent[1] + ent[2] == 2 * NJT:
                        prev_norm = seg_finish(ent[3], ent[4], ent[5], prev_norm)
                do_norm(*prev_norm, on_act=True)

            # ---- phase 3: output projection ----
            with tc.tile_pool(name="ops", bufs=2, space=bass.MemorySpace.PSUM) as ops:
                for m in range(2):
                    ps = ops.tile([128, LQ], F32, tag="o", name="pso")
                    for k in range(3):
                        for n in range(2):
                            nc.tensor.matmul(
                                ps[:, 512 * n : 512 * (n + 1)],
                                wo[k][:, 128 * m : 128 * (m + 1)],
                                attn[k][:, 512 * n : 512 * (n + 1)],
                                start=(k == 0),
                                stop=False,
                            )
                    for half in range(2):
                        hr = slice(64 * half, 64 * (half + 1))
                        for n in range(2):
                            nc.tensor.matmul(
                                ps[:, 512 * n : 512 * (n + 1)],
                                wo[3][hr, 128 * m : 128 * (m + 1)],
                                attn[3][hr, 512 * n : 512 * (n + 1)],
                                start=False,
                                stop=(half == 1),
                            )
                    osb = op.tile([128, LQ], BF16, tag="osb", name="osb")
                    for n in range(2):
                        cols = slice(512 * n, 512 * (n + 1))
                        if m == 0:
                            nc.scalar.add(osb[:, cols], ps[:, cols], bias[m][:])
                        else:
                            nc.vector.tensor_scalar_add(
                                osb[:, cols], ps[:, cols], bias[m][:]
                            )
                        nc.sync.dma_start(
                            out_d.ap()[128 * m : 128 * (m + 1), cols], osb[:, cols]
                        )

    nc.compile()
    return nc


def get_nc():
    if "nc" not in _cached:
        _cached["nc"] = build_nc()
    return _cached["nc"]


def make_in_maps(x, w_qkv, w_out, b_out):
    import ml_dtypes

    bf16 = ml_dtypes.bfloat16
    wqkvT = np.ascontiguousarray(w_qkv.T.astype(bf16))
    woutT = np.ascontiguousarray(w_out.T.astype(bf16))
    bias = np.ascontiguousarray(b_out.astype(np.float32).reshape(C, 1))
    in_maps = []
    for i in range(8):
        b, qh = i // 2, i % 2
        xb = np.ascontiguousarray(x[b].astype(bf16))
        xq = np.ascontiguousarray(xb[:, qh * LQ : (qh + 1) * LQ])
        in_maps.append(
            {"x": xb, "xq": xq, "wqkvT": wqkvT, "woutT": woutT, "bias": bias}
        )
    return in_maps


def assemble(results):
    out = np.empty((B, C, L), dtype=np.float32)
    for i in range(8):
        b, qh = i // 2, i % 2
        out[b][:, qh * LQ : (qh + 1) * LQ] = np.asarray(
            results[i]["out"], dtype=np.float32
        )
    return out


def kernel(x, w_qkv, w_out, b_out):
    x = np.asarray(x, dtype=np.float32)
    w_qkv = np.asarray(w_qkv, dtype=np.float32)
    w_out = np.asarray(w_out, dtype=np.float32)
    b_out = np.asarray(b_out, dtype=np.float32)
    assert x.shape == (B, C, L), x.shape
    nc = get_nc()
    in_maps = make_in_maps(x, w_qkv, w_out, b_out)
    res = run_bass_kernel_spmd(nc, in_maps, list(range(8)), trace=False)
    return assemble(res.results)


# revision 54
# speedup vs baseline: 1.1989x; 1.0207x over previous
"""Multi-head attention (b=4, c=256, l=2048, 8 heads x 64) on 8 TRN2 NeuronCores.

Sharding: core i handles batch b = i//2 and query half qh = i%2 (1024 queries),
computing all 8 heads over the full 2048-key context. Outputs are disjoint
[256, 1024] slabs -> host-side concat only, no collectives.

Per-core kernel (bf16 matmuls at 1 cycle/row, fp32 PSUM accumulate):
  1. Q = Wq @ xq, K = Wk @ x, VT = (Wv @ x)^T projected directly into a
     transposed [l-tile 128, 8 heads x 65] layout whose ones column (64)
     makes the PV matmul emit the softmax denominator for free.
  2. Attention runs in 8 SEGMENTS = (head-pair, 512-query-half). Scores go
     to [128, 1536] psum tiles whose 3 x 512-column chunks interleave BOTH
     heads of the pair (exp is elementwise, mixed-head tiles are fine) --
     amortizing the ~310-cycle ACTIVATE overhead. PSUM: 2 x 3-bank score
     slots (double-buffered) + 2 x 1-bank [65, 512] PV accumulators = 8.
  3. Three nested software pipelines with no drain points: PV flushes lag
     the QK/exp stream by one tile, the normalization chain (psum-freeing
     copy, reciprocal, GpSimd partition-broadcast, numerator multiply) lags
     by one segment, and BOTH lags are carried ACROSS segment boundaries so
     the 134 us exp stream runs with zero gaps. The last segment's
     reciprocal uses exp(-ln) on the then-idle ScalarE.
  4. out = WoutT^T . attn + bias, DMA to DRAM as bf16 (host upcasts).

Measured: ScalarE and TensorE both ~147 us busy (dual roofline: QK+PV
stream 2 matmul columns per score element at 2.4 GHz vs exp's 1 element per
lane at 1.2 GHz -> identical ~109 us floors). Span ~197 us = ~42 us lead-in
(NEFF preamble + 8-core DMA contention + projection FIFO, deadline-pinned)
+ 134 us gapless exp stream + ~18 us tail incl ~7 us fixed postamble.
"""

import sys

if "/opt/trn_rl_repo" not in sys.path:
    sys.path.insert(0, "/opt/trn_rl_repo")

import numpy as np

import concourse.bass as bass
import concourse.mybir as mybir
import concourse.tile as tile
from concourse import bacc
from concourse.bass_utils import run_bass_kernel_spmd

F32 = mybir.dt.float32
F32R = mybir.dt.float32r
BF16 = mybir.dt.bfloat16
EXP = mybir.ActivationFunctionType.Exp
LOG = mybir.ActivationFunctionType.Ln
MULT = mybir.AluOpType.mult

B, C, L = 4, 256, 2048
H, D = 8, 64
HID = H * D  # 512
LQ = L // 2  # 1024 queries per core
NJT = L // 128  # 16 key tiles
SCALE = D**-0.5

_cached = {}


def r(ap):
    return ap


def build_nc():
    nc = bacc.Bacc(
        "TRN2",
        target_bir_lowering=False,
        debug=False,
        enable_asserts=False,
        num_devices=8,
    )
    x_d = nc.dram_tensor("x", [C, L], BF16, kind="ExternalInput")
    xq_d = nc.dram_tensor("xq", [C, LQ], BF16, kind="ExternalInput")
    wq_d = nc.dram_tensor("wqkvT", [C, 3 * HID], BF16, kind="ExternalInput")
    wo_d = nc.dram_tensor("woutT", [HID, C], BF16, kind="ExternalInput")
    bias_d = nc.dram_tensor("bias", [C, 1], F32, kind="ExternalInput")
    out_d = nc.dram_tensor("out", [C, LQ], BF16, kind="ExternalOutput")

    with tile.TileContext(nc) as tc:
        with (
            tc.tile_pool(name="const", bufs=1) as cp,
            tc.tile_pool(name="epool", bufs=9) as ep,
            tc.tile_pool(name="rpool", bufs=2) as rp,
            tc.tile_pool(name="opool", bufs=2) as op,
        ):
            # ---- persistent SBUF tensors ----
            xb = [cp.tile([128, L], BF16, tag=f"xb{k}", name=f"xb{k}") for k in range(2)]
            xq = [cp.tile([128, LQ], BF16, tag=f"xq{k}", name=f"xq{k}") for k in range(2)]
            wq = [cp.tile([128, 3 * HID], BF16, tag=f"wq{k}", name=f"wq{k}") for k in range(2)]
            wo = [cp.tile([128, C], BF16, tag=f"wo{k}", name=f"wo{k}") for k in range(4)]
            bias = [cp.tile([128, 1], F32, tag=f"bias{k}", name=f"bias{k}") for k in range(2)]
            Qs = [cp.tile([128, LQ], BF16, tag=f"Q{m}", name=f"Q{m}") for m in range(4)]
            Ks = [cp.tile([128, L], BF16, tag=f"K{m}", name=f"K{m}") for m in range(4)]
            VT = [cp.tile([128, H, D + 1], BF16, tag=f"VT{t}", name=f"VT{t}") for t in range(NJT)]
            attn = [cp.tile([128, LQ], BF16, tag=f"attn{m}", name=f"attn{m}") for m in range(4)]
            acc = [cp.tile([128, LQ], F32, tag=f"acc{m}", name=f"acc{m}") for m in range(2)]
            dum = cp.tile([1, 16], F32, tag="dum", name="dum")
            dumo = cp.tile([1, 16], F32, tag="dumo", name="dumo")
            nc.gpsimd.memset(dum[:], 1.0)
            nc.scalar.activation(dumo[:], dum[:], LOG)
            nc.scalar.activation(dumo[:], dum[:], EXP)

            # ---- DMA inputs ----
            nc.sync.dma_start(wq[0][:, 0:512], wq_d.ap()[0:128, 0:512])
            nc.gpsimd.dma_start(wq[1][:, 0:512], wq_d.ap()[128:256, 0:512])
            nc.sync.dma_start(wq[0][:, 512:640], wq_d.ap()[0:128, 512:640])
            nc.gpsimd.dma_start(wq[1][:, 512:640], wq_d.ap()[128:256, 512:640])
            for k in range(2):
                rows = slice(128 * k, 128 * (k + 1))
                nc.scalar.dma_start(xq[k][:, 0:512], xq_d.ap()[rows, 0:512])
            for k in range(2):
                rows = slice(128 * k, 128 * (k + 1))
                nc.scalar.dma_start(xq[k][:, 512:1024], xq_d.ap()[rows, 512:1024])
            for k in range(2):
                rows = slice(128 * k, 128 * (k + 1))
                nc.sync.dma_start(xb[k][:, 0:1024], x_d.ap()[rows, 0:1024])
            for k in range(2):
                rows = slice(128 * k, 128 * (k + 1))
                nc.gpsimd.dma_start(xb[k][:, 1024:2048], x_d.ap()[rows, 1024:2048])
            nc.scalar.dma_start(wq[0][:, 640:1536], wq_d.ap()[0:128, 640:1536])
            nc.gpsimd.dma_start(wq[1][:, 640:1536], wq_d.ap()[128:256, 640:1536])
            for k in range(4):
                nc.sync.dma_start(wo[k][:], wo_d.ap()[128 * k : 128 * (k + 1), :])
            for k in range(2):
                rows = slice(128 * k, 128 * (k + 1))
                nc.gpsimd.dma_start(bias[k][:], bias_d.ap()[rows, :])

            # ---- phase 1: projections ----
            with (
                tc.tile_pool(name="pps", bufs=2, space=bass.MemorySpace.PSUM) as pps,
                tc.tile_pool(name="vps", bufs=2, space=bass.MemorySpace.PSUM) as vps,
            ):
                def q_proj(m):
                    ps = pps.tile([128, LQ], F32, tag="proj", name="ps")
                    for k in range(2):
                        for n in range(2):
                            nc.tensor.matmul(
                                ps[:, 512 * n : 512 * (n + 1)],
                                wq[k][:, 128 * m : 128 * (m + 1)],
                                xq[k][:, 512 * n : 512 * (n + 1)],
                                start=(k == 0),
                                stop=(k == 1),
                            )
                    if m == 0:
                        for n in range(2):
                            cols = slice(512 * n, 512 * (n + 1))
                            nc.vector.tensor_scalar_mul(
                                Qs[m][:, cols], ps[:, cols], SCALE
                            )
                    else:
                        nc.scalar.mul(Qs[m][:], ps[:], SCALE)

                def k_proj(m):
                    for lh in range(2):
                        ps = pps.tile([128, LQ], F32, tag="proj", name="ps")
                        for k in range(2):
                            for n in range(2):
                                nc.tensor.matmul(
                                    ps[:, 512 * n : 512 * (n + 1)],
                                    wq[k][:, HID + 128 * m : HID + 128 * (m + 1)],
                                    xb[k][:, 1024 * lh + 512 * n : 1024 * lh + 512 * (n + 1)],
                                    start=(k == 0),
                                    stop=(k == 1),
                                )
                        if m == 0:
                            for n in range(2):
                                nc.scalar.copy(
                                    Ks[m][
                                        :,
                                        1024 * lh + 512 * n : 1024 * lh + 512 * (n + 1),
                                    ],
                                    ps[:, 512 * n : 512 * (n + 1)],
                                )
                        else:
                            nc.vector.tensor_copy(
                                Ks[m][:, 1024 * lh : 1024 * (lh + 1)], ps[:]
                            )

                def vt_proj(t):
                    ps = vps.tile([128, HID], F32, tag="vproj", name="psv")
                    for k in range(2):
                        nc.tensor.matmul(
                            ps[:],
                            xb[k][:, 128 * t : 128 * (t + 1)],
                            wq[k][:, 2 * HID : 3 * HID],
                            start=(k == 0),
                            stop=(k == 1),
                        )
                    nc.vector.tensor_copy(
                        VT[t][:, :, 0:D], ps[:].rearrange("p (h c) -> p h c", h=H)
                    )
                    nc.gpsimd.memset(VT[t][:, :, D : D + 1], 1.0)

                q_proj(0)
                k_proj(0)
                vt_proj(0)
                vt_proj(1)
                # hoist the first 3 score tiles of segment 0 into the
                # phase-1 window: QK+exp run here (filling the otherwise
                # idle ScalarE during the DMA lead-in); their PV flushes
                # drain through the main loop's pending FIFO.
                hoist = []
                for hc in range(6):
                    ps = pps.tile([128, 1024], F32, tag="proj", name="hqk")
                    for ci in range(2):
                        jt, s = divmod(2 * hc + ci, 2)
                        nc.tensor.matmul(
                            ps[:, 512 * ci : 512 * (ci + 1)],
                            Ks[0][64 * s : 64 * (s + 1), 128 * jt : 128 * (jt + 1)],
                            Qs[0][64 * s : 64 * (s + 1), 0:512],
                            start=True,
                            stop=True,
                        )
                    Eh = ep.tile([128, 1024], BF16, tag="e", name="Eh")
                    nc.scalar.activation(Eh[:], ps[:], EXP)
                    hoist.append((Eh, 2 * hc, 2))
                q_proj(1)
                k_proj(1)
                vt_proj(2)
                vt_proj(3)
                q_proj(2)
                k_proj(2)
                q_proj(3)
                k_proj(3)
                for t in range(4, NJT):
                    vt_proj(t)

            # ---- phase 2: attention ----
            # Segments = (head-pair, 512-query-half). Scores go to [128,
            # 1536] psum tiles (3 chunks of 512 cols, both heads interleaved
            # - exp is elementwise so mixed-head tiles are fine). This
            # amortizes the ACTIVATE overhead (1540 ns / 3 chunks vs 1114/2)
            # while the PV accumulators shrink to [65, 512] = 1 bank each:
            # qk 2x3 banks + pv 2x1 = 8 banks exactly. PV lags one tile; the
            # normalization chain runs one SEGMENT behind so the PV slots
            # free after two fast copies.
            with (
                tc.tile_pool(name="qkps", bufs=2, space=bass.MemorySpace.PSUM) as qkps,
                tc.tile_pool(name="pvps", bufs=1, space=bass.MemorySpace.PSUM) as pvps,
            ):
                def do_norm(pons, p, ih, on_act):
                    cols = slice(512 * ih, 512 * (ih + 1))
                    for s in (0, 1):
                        rec = rp.tile([1, 512], F32, tag="rec", name="rec")
                        if on_act:
                            lnd = rp.tile([1, 512], F32, tag="lnd", name="lnd")
                            nc.scalar.activation(lnd[:], pons[s][D : D + 1, :], LOG)
                            nc.scalar.activation(rec[:], lnd[:], EXP, scale=-1.0)
                        else:
                            nc.vector.reciprocal(rec[:], pons[s][D : D + 1, :])
                        rbc = rp.tile([64, 512], F32, tag="rbc", name="rbc")
                        nc.gpsimd.partition_broadcast(rbc[:], rec[:])
                        nc.vector.tensor_tensor(
                            attn[p][64 * s : 64 * (s + 1), cols],
                            pons[s][0:D, :],
                            rbc[:],
                            MULT,
                        )

                def pv_flush(E, c0, nch, po, p):
                    for ci in range(nch):
                        jt, s = divmod(c0 + ci, 2)
                        nc.tensor.matmul(
                            po[s][:],
                            VT[jt][:, 2 * p + s, :],
                            E[:, 512 * ci : 512 * (ci + 1)],
                            start=(jt == 0),
                            stop=(jt == NJT - 1),
                        )

                def seg_finish(po, p, ih, prev_norm):
                    # copy num+den out (frees the PV psum slots), then run
                    # the previous segment's normalization chain
                    pons = [
                        rp.tile([D + 1, 512], F32, tag="pon", name="pon", bufs=4)
                        for s in (0, 1)
                    ]
                    for s in (0, 1):
                        nc.vector.tensor_copy(pons[s][:], po[s][:])
                    if prev_norm is not None:
                        do_norm(*prev_norm, on_act=False)
                    return (pons, p, ih)

                # The PV-flush lag is carried ACROSS segment boundaries: the
                # last tile of segment s flushes inside segment s+1's first
                # iteration, so the PE/ACT pipeline never drains mid-stream.
                pending = []  # FIFO of (E, c0, nch, po, p, ih)
                prev_norm = None
                for seg in range(8):
                    p, ih = divmod(seg, 2)
                    Qh = [
                        Qs[p][64 * s : 64 * (s + 1), 512 * ih : 512 * (ih + 1)]
                        for s in (0, 1)
                    ]
                    Kh = [Ks[p][64 * s : 64 * (s + 1), :] for s in (0, 1)]
                    po = [
                        pvps.tile([D + 1, 512], F32, tag=f"pv{s}", name=f"po{s}")
                        for s in (0, 1)
                    ]
                    c0 = 0
                    if seg == 0:
                        pending = [(E, hc0, hn, po, p, ih) for (E, hc0, hn) in hoist]
                        c0 = 12
                    while c0 < 2 * NJT:
                        nch = min(3, 2 * NJT - c0)
                        ps = qkps.tile([128, 512 * nch], F32, tag="qk", name="psqk")
                        for ci in range(nch):
                            jt, s = divmod(c0 + ci, 2)
                            nc.tensor.matmul(
                                ps[:, 512 * ci : 512 * (ci + 1)],
                                Kh[s][:, 128 * jt : 128 * (jt + 1)],
                                Qh[s][:],
                                start=True,
                                stop=True,
                            )
                        if pending:
                            ent = pending.pop(0)
                            pv_flush(*ent[:5])
                            if ent[1] + ent[2] == 2 * NJT:
                                prev_norm = seg_finish(ent[3], ent[4], ent[5], prev_norm)
                        E = ep.tile([128, 512 * nch], BF16, tag="e", name="E")
                        nc.scalar.activation(E[:], ps[:], EXP)
                        pending.append((E, c0, nch, po, p, ih))
                        c0 += nch
                for ent in pending:
                    pv_flush(*ent[:5])
                    if ent[1] + ent[2] == 2 * NJT:
                        prev_norm = seg_finish(ent[3], ent[4], ent[5], prev_norm)
                do_norm(*prev_norm, on_act=True)

            # ---- phase 3: output projection ----
            with tc.tile_pool(name="ops", bufs=2, space=bass.MemorySpace.PSUM) as ops:
                for m in range(2):
                    ps = ops.tile([128, LQ], F32, tag="o", name="pso")
                    for k in range(3):
                        for n in range(2):
                            nc.tensor.matmul(
                                ps[:, 512 * n : 512 * (n + 1)],
                                wo[k][:, 128 * m : 128 * (m + 1)],
                                attn[k][:, 512 * n : 512 * (n + 1)],
                                start=(k == 0),
                                stop=False,
                            )
                    for half in range(2):
                        hr = slice(64 * half, 64 * (half + 1))
                        for n in range(2):
                            nc.tensor.matmul(
                                ps[:, 512 * n : 512 * (n + 1)],
                                wo[3][hr, 128 * m : 128 * (m + 1)],
                                attn[3][hr, 512 * n : 512 * (n + 1)],
                                start=False,
                                stop=(half == 1),
                            )
                    osb = op.tile([128, LQ], BF16, tag="osb", name="osb")
                    for n in range(2):
                        cols = slice(512 * n, 512 * (n + 1))
                        if m == 0:
                            nc.scalar.add(osb[:, cols], ps[:, cols], bias[m][:])
                        else:
                            nc.vector.tensor_scalar_add(
                                osb[:, cols], ps[:, cols], bias[m][:]
                            )
                        nc.sync.dma_start(
                            out_d.ap()[128 * m : 128 * (m + 1), cols], osb[:, cols]
                        )

    nc.compile()
    return nc


def get_nc():
    if "nc" not in _cached:
        _cached["nc"] = build_nc()
    return _cached["nc"]


def make_in_maps(x, w_qkv, w_out, b_out):
    import ml_dtypes

    bf16 = ml_dtypes.bfloat16
    wqkvT = np.ascontiguousarray(w_qkv.T.astype(bf16))
    woutT = np.ascontiguousarray(w_out.T.astype(bf16))
    bias = np.ascontiguousarray(b_out.astype(np.float32).reshape(C, 1))
    in_maps = []
    for i in range(8):
        b, qh = i // 2, i % 2
        xb = np.ascontiguousarray(x[b].astype(bf16))
        xq = np.ascontiguousarray(xb[:, qh * LQ : (qh + 1) * LQ])
        in_maps.append(
            {"x": xb, "xq": xq, "wqkvT": wqkvT, "woutT": woutT, "bias": bias}
        )
    return in_maps


def assemble(results):
    out = np.empty((B, C, L), dtype=np.float32)
    for i in range(8):
        b, qh = i // 2, i % 2
        out[b][:, qh * LQ : (qh + 1) * LQ] = np.asarray(
            results[i]["out"], dtype=np.float32
        )
    return out


def kernel(x, w_qkv, w_out, b_out):
    x = np.asarray(x, dtype=np.float32)
    w_qkv = np.asarray(w_qkv, dtype=np.float32)
    w_out = np.asarray(w_out, dtype=np.float32)
    b_out = np.asarray(b_out, dtype=np.float32)
    assert x.shape == (B, C, L), x.shape
    nc = get_nc()
    in_maps = make_in_maps(x, w_qkv, w_out, b_out)
    res = run_bass_kernel_spmd(nc, in_maps, list(range(8)), trace=False)
    return assemble(res.results)
